# revision 28
# baseline (speedup 1.0000x reference)
"""Trainium2 Bass kernel for the Backflow module.

Math (B=16, N=512, DIM=3, H=32):
  out[b,i,:] = sum_j eta(||x_bi - x_bj||) * (x_bi - x_bj)  +  mu(||x_bi||) * x_bi
where eta/mu are 1->H->1 tanh MLPs. The reference's eye()/diagonal correction
cancels exactly (eta(0)*(x_i - x_i) = 0 in the matrix form below).

Sharding: data-parallel over batch, 2 batches per core on 8 cores.

eta and mu are univariate scalar functions and the rel-err budget (2e-2)
is large, so we fit cheap surrogates at call time from the actual weights,
both in u = d^2 (no sqrt anywhere; exp/identity/copy live in one ACT
table set -> a single table load):

  t[i,j] = 2*d_ij^2/umax - 1 comes straight out of the PE: the d^2
  matmul carries two extra rows ([-2sx | s | s*n2_i - 1] stationary x
  [x | n2_j | 1] moving, f32r) so PSUM holds t directly.

  M[i,j] := -eta(d_ij) - c0 evaluated two ways on disjoint column regions
  of the packed strip:
   A-region (ACT+PE): sum_m c_m exp(g_m (t+1)) - META exp ACT passes
     reading PSUM directly, |c_m| folded into the bias, sign via
     +/-identity f32r stationaries accumulated on the PE into PSUM;
     one DVE copy -> bf16 M tile.
   B-region (DVE): one ACT copy of the t columns to SBUF, then monomial
     Horner for P(t) - c0 via stock scalar_tensor_tensor ops.
  The split ratio load-balances ACT vs DVE; end-to-end fit error ~5e-4
  (exp) / ~2.5e-3 (poly deg 11) on top of ~1e-3 of bf16/f32r noise.

  The shared constant c0 is folded into the finalize for free:
  out_c[j] = (P'_c[j] + c0*X_c) - x_c[j]*(Q'[j] + c0*N), X_c = sum_i x_c[i].

  mu(||x_i||) = c0' + sum_m c'_m exp(-b_m n_i^2): ONE ACT exp pass on a
  [MU, N] broadcast of n^2 (per-partition scale), folded into the Q rows
  of the PSUM contraction with a negated bf16 stationary (so e_n costs
  no DVE work).

Per-core layout: i on partitions (4 chunks of 128), j on the free dim.
Symmetry eta(d_ij) = eta(d_ji): compute only block-triangular strips
(chunk I covers j in [128*I, 512)), packed to [128, 1280] with
bank-aligned chunk offsets (order 0,1,3,2) so every matmul output stays
inside a PSUM bank.

Row sums via PE contractions (3-wide ones / x stationaries in bf16, M
blocks moving in bf16 = 1 cyc/row). Direct blocks give the (J,*) rows,
PE-transposed blocks (bf16) give the reflected (I,*) rows; the
transposed blocks return to SBUF via DMA (no engine time).
"""

import sys

sys.path.insert(0, "/opt/trn_rl_repo")

import numpy as np
from contextlib import ExitStack

B, N, DIM, H = 16, 512, 3, 32
NCORES = 8
BPC = B // NCORES  # batches per core
P = 128
NCHUNK = N // P  # 4
WIDTHS = [N - P * I for I in range(NCHUNK)]  # [512, 384, 256, 128]
# bank-aligned packing of the block-triangular strips (chunk order 0,1,3,2):
# every chunk's [128, W] matmul output stays inside 2KB PSUM banks.
OFFS = [0, 512, 1024, 896]
NPACK = sum(WIDTHS)  # 1280

DEG = 11  # B-region polynomial degree
META = 6  # A-region exp basis size
MU = 12  # mu exp-basis size (incl. the g=0 constant term)
ASPLIT = 768  # packed columns [0, ASPLIT) on ACT path, rest on DVE path
# f32r accumulate matmuls want >=256-wide splits that respect PSUM banks
ASEGS = [(0, 512), (512, 256)]
assert ASEGS[-1][0] + ASEGS[-1][1] == ASPLIT

LAST_RESULT = None


def _spread_sync_waits(nc):
    """The pinned walrus rejects instructions carrying more than one sync wait
    ('Too many sync wait commands'). Engines execute their instruction streams
    in order, so hoist all-but-one wait of any such instruction onto same-engine
    NoOps inserted directly before it — semantically identical ordering."""
    from concourse import mybir

    n_added = 0
    for bb in nc.main_func.blocks:
        insts = bb.instructions
        i = 0
        while i < len(insts):
            inst = insts[i]
            si = getattr(inst, "sync_info", None)
            waits = list(si.on_wait) if si is not None and si.on_wait else []
            if len(waits) > 1:
                si.on_wait = waits[-1:]
                for k, w in enumerate(waits[:-1]):
                    nop = mybir.InstNoOp(
                        name=f"{inst.name}-wspread{k}",
                        sync_info=mybir.SyncInfo(on_wait=[w], on_update=[]),
                        engine=inst.engine,
                        bass_nofuse=True,
                    )
                    insts.insert(i + k, nop)
                    n_added += 1
                i += len(waits) - 1
            i += 1
    return n_added


def _eta_fn(d, w1, b1, w2, b2):
    return np.tanh(d[..., None] * w1[0] + b1) @ w2[:, 0] + b2[0]


def _fit_surrogates(x, eta_w1, eta_b1, eta_w2, eta_b2):
    """Global fits of f(t) = -eta(sqrt(u)), t = 2u/umax - 1:
    poly (ascending monomial coeffs, deg DEG) and exp basis
    f - c0 ~= sum_m c_m exp(g_m (t+1)). Returns (s, pc, gam, ce)."""
    x = x.astype(np.float64)
    n2 = (x**2).sum(-1)  # [B, N]
    rng = np.random.default_rng(0)
    umax = 0.0
    samples = []
    for b in range(B):
        G = x[b] @ x[b].T
        Ub = np.maximum(n2[b][:, None] + n2[b][None, :] - 2 * G, 0.0)
        umax = max(umax, float(Ub.max()))
        idx = rng.choice(N * N, 16384, replace=False)
        samples.append(Ub.reshape(-1)[idx])
    umax = umax * 1.002 + 1e-6
    uu = np.concatenate(samples)
    ug = np.linspace(0.0, umax, 2000)
    ufit = np.concatenate([uu, ug])
    w = np.concatenate(
        [np.sqrt(np.sqrt(uu) + 0.1), 3.0 * np.sqrt(np.sqrt(ug) + 0.1)]
    )
    tfit = 2.0 * ufit / umax - 1.0
    yfit = -_eta_fn(np.sqrt(ufit), eta_w1, eta_b1, eta_w2, eta_b2)
    import numpy.polynomial.chebyshev as Ch

    cf = Ch.chebfit(tfit, yfit, DEG, w=w)
    pc = Ch.cheb2poly(cf)  # ascending monomial coeffs in t
    c0 = float(pc[0])
    # exp basis on the residual target f - c0, no free constant
    gam = -np.geomspace(0.08, 48.0, META)  # exponents per (t+1) unit
    A = np.exp((tfit[:, None] + 1.0) * gam[None, :])
    Aw = A * w[:, None]
    ce, *_ = np.linalg.lstsq(Aw, (yfit - c0) * w, rcond=None)
    s = 2.0 / umax
    return float(s), pc.astype(np.float64), gam, ce


def _fit_mu_exp(n2_all, mu_w1, mu_b1, mu_w2, mu_b2):
    """Fit mu(sqrt(u)) ~= sum_m c_m exp(-g_m u) on the actual n^2 values
    (the exact evaluation points). g_0 = 0 supplies the constant term."""
    us = np.sort(n2_all.reshape(-1).astype(np.float64))
    n2max = float(us[-1]) * 1.001 + 1e-9
    g = np.concatenate([[0.0], np.geomspace(0.125, 96.0, MU - 1) / n2max])
    A = np.exp(-us[:, None] * g[None, :])
    y = _eta_fn(np.sqrt(us), mu_w1, mu_b1, mu_w2, mu_b2)
    w = np.sqrt(np.sqrt(us) + 0.1)
    Aw = A * w[:, None]
    AtA = Aw.T @ Aw + 1e-10 * len(us) * np.eye(MU)
    c = np.linalg.solve(AtA, Aw.T @ (y * w))
    return g.astype(np.float64), c.astype(np.float64)


def _build_program(poly_pc, eta_gam, eta_ce):
    import concourse.bass as bass
    import concourse.tile as tile
    from concourse import mybir

    f32 = mybir.dt.float32
    f32r = mybir.dt.float32r
    bf16 = mybir.dt.bfloat16
    AF = mybir.ActivationFunctionType
    OP = mybir.AluOpType

    pc = [float(v) for v in poly_pc]  # ascending, len DEG+1
    c0 = pc[0]
    ea_scale = [float(g) for g in eta_gam]
    ea_sign = [1.0 if c > 0 else -1.0 for c in eta_ce]

    DR = DIM + 2  # d^2 matmul rows: x(3), n2, ones

    nc = bass.Bass()
    xTn_d = nc.dram_tensor("xTn", [DR, BPC, N], f32, kind="ExternalInput")
    statd_d = nc.dram_tensor("statd", [DR, BPC, NCHUNK, P], f32, kind="ExternalInput")
    statx6_d = nc.dram_tensor("statx6", [P, BPC, NCHUNK, 2 * DIM], bf16, kind="ExternalInput")
    identb_d = nc.dram_tensor("identb", [P, P], bf16, kind="ExternalInput")
    ident_d = nc.dram_tensor("ident", [P, P], f32, kind="ExternalInput")
    unrep_d = nc.dram_tensor("unrep", [MU, BPC, N], f32, kind="ExternalInput")
    negbeta_d = nc.dram_tensor("negbeta", [MU, 1], f32, kind="ExternalInput")
    muAb_d = nc.dram_tensor("muAb", [MU, DIM], bf16, kind="ExternalInput")
    c0x_d = nc.dram_tensor("c0x", [DIM, BPC], f32, kind="ExternalInput")
    eab_d = nc.dram_tensor("eab", [P, META + 1], f32, kind="ExternalInput")
    out_d = nc.dram_tensor("out", [BPC, DIM, N], f32, kind="ExternalOutput")

    with tile.TileContext(nc) as tc, ExitStack() as ctx:
        singles = ctx.enter_context(tc.tile_pool(name="singles", bufs=1))
        tpool = ctx.enter_context(tc.tile_pool(name="tpool", bufs=2))
        hpool = ctx.enter_context(tc.tile_pool(name="hpool", bufs=2))
        hsp = ctx.enter_context(tc.tile_pool(name="hsp", bufs=8))
        mpool = ctx.enter_context(tc.tile_pool(name="mpool", bufs=2))
        atp = ctx.enter_context(tc.tile_pool(name="atp", bufs=4))
        xbp = ctx.enter_context(tc.tile_pool(name="xbp", bufs=2))
        hmup = ctx.enter_context(tc.tile_pool(name="hmup", bufs=2))
        finp = ctx.enter_context(tc.tile_pool(name="finp", bufs=2))
        orp = ctx.enter_context(tc.tile_pool(name="orp", bufs=2))
        psd2 = ctx.enter_context(tc.tile_pool(name="psd2", bufs=1, space="PSUM"))
        psacc = ctx.enter_context(tc.tile_pool(name="psacc", bufs=1, space="PSUM"))
        psout = ctx.enter_context(tc.tile_pool(name="psout", bufs=1, space="PSUM"))
        pstr = ctx.enter_context(tc.tile_pool(name="pstr", bufs=1, space="PSUM"))

        # ---- inputs; d^2-path tensors first (they gate the first matmul) ----
        statd_sb = singles.tile([DR, BPC, NCHUNK, P], f32)
        nc.gpsimd.dma_start(out=statd_sb[:], in_=statd_d[:])
        xTn_sb = singles.tile([DR, BPC, N], f32)
        nc.gpsimd.dma_start(out=xTn_sb[:], in_=xTn_d[:])
        eab_sb = singles.tile([P, META + 1], f32)
        nc.gpsimd.dma_start(out=eab_sb[:], in_=eab_d[:])
        ident_sb = singles.tile([P, P], f32)
        nc.gpsimd.dma_start(out=ident_sb[:], in_=ident_d[:])
        statx6b = singles.tile([P, BPC, NCHUNK, 2 * DIM], bf16)
        nc.gpsimd.dma_start(out=statx6b[:], in_=statx6_d[:])
        identb = singles.tile([P, P], bf16)
        nc.gpsimd.dma_start(out=identb[:], in_=identb_d[:])
        unrep_sb = singles.tile([MU, BPC, N], f32)
        nc.gpsimd.dma_start(out=unrep_sb[:], in_=unrep_d[:])
        negbeta_sb = singles.tile([MU, 1], f32)
        nc.gpsimd.dma_start(out=negbeta_sb[:], in_=negbeta_d[:])
        muAb = singles.tile([MU, DIM], bf16)
        nc.gpsimd.dma_start(out=muAb[:], in_=muAb_d[:])
        c0x_sb = singles.tile([DIM, BPC], f32)
        nc.gpsimd.dma_start(out=c0x_sb[:], in_=c0x_d[:])

        # f32r conversions for the d^2 matmul operands (DVE rounds)
        statd_r = singles.tile([DR, BPC, NCHUNK, P], f32r)
        nc.vector.tensor_copy(statd_r[:], statd_sb[:])
        xTn_r = singles.tile([DR, BPC, N], f32r)
        nc.vector.tensor_copy(xTn_r[:], xTn_sb[:])
        # +/- identity in f32r for the sign of exp-basis coefficients
        identr = singles.tile([P, P], f32r)
        nc.vector.tensor_copy(identr[:], ident_sb[:])
        nidentr = singles.tile([P, P], f32r)
        nc.vector.tensor_scalar_mul(out=nidentr[:], in0=ident_sb[:], scalar1=-1.0)
        xb16 = {}
        for b in range(BPC):
            xb = xbp.tile([DIM, N], bf16, tag="xb")
            nc.vector.tensor_copy(xb[:], xTn_sb[0:DIM, b, :])
            xb16[b] = xb

        # ---- t strips straight from the PE ----
        # psum[i,j] = s*d2_ij - 1 = t  (stationary rows [-2sx | s | s*n2_i-1],
        # moving rows [x | n2_j | 1])
        def emit_d2(b):
            tps = psd2.tile([P, NPACK], f32, tag="t")
            for I in range(NCHUNK):
                nc.tensor.matmul(
                    tps[:, OFFS[I] : OFFS[I] + WIDTHS[I]],
                    statd_r[:, b, I, :],
                    xTn_r[:, b, P * I : N],
                    start=True,
                    stop=True,
                    skip_group_check=True,
                )
            return tps

        def emit_expacc(b, tps):
            """A-region: META exp passes on ACT reading PSUM t directly,
            +/-I f32r accumulate on PE."""
            acc = psacc.tile([P, ASPLIT], f32, tag="acc")
            for m in range(META):
                hs = hsp.tile([P, ASPLIT], f32r, tag="hs")
                nc.scalar.activation(
                    hs[:],
                    tps[:, 0:ASPLIT],
                    AF.Exp,
                    scale=ea_scale[m],
                    bias=eab_sb[:, m : m + 1],
                )
                stat = identr if ea_sign[m] > 0 else nidentr
                for off, w in ASEGS:
                    nc.tensor.matmul(
                        acc[:, off : off + w],
                        stat[:],
                        hs[:, off : off + w],
                        start=(m == 0),
                        stop=(m == META - 1),
                        skip_group_check=True,
                    )
            return acc

        def emit_tcopy(b, tps):
            """B-region t columns PSUM -> SBUF (one ACT copy)."""
            tB = tpool.tile([P, NPACK - ASPLIT], f32, tag="t")
            nc.scalar.copy(tB[:], tps[:, ASPLIT:NPACK])
            return tB

        def emit_horner(b, tB, Mt):
            """B-region: monomial Horner for P(t) - c0 on DVE (stock ops):
            g = c_deg * t; then g = (g + c_j) * t for j = deg-1 .. 1."""
            g = hpool.tile([P, NPACK - ASPLIT], f32, tag="h")
            nc.vector.tensor_scalar_mul(out=g[:], in0=tB[:], scalar1=pc[DEG])
            gap = g[:]
            for j in range(DEG - 1, 0, -1):
                if j == 1:
                    dst_ap = Mt[:, ASPLIT:NPACK]
                else:
                    dst = hpool.tile([P, NPACK - ASPLIT], f32, tag="h")
                    dst_ap = dst[:]
                nc.vector.scalar_tensor_tensor(
                    out=dst_ap,
                    in0=gap,
                    scalar=pc[j],
                    in1=tB[:],
                    op0=OP.add,
                    op1=OP.mult,
                )
                gap = dst_ap

        def emit_merge(b, acc, Mt):
            nc.vector.tensor_copy(Mt[:, 0:ASPLIT], acc[:])

        def blkoff(I, J):
            return OFFS[I] + (J - I) * P

        def emit_contract(b, Mt):
            at_sb = {}
            tps_l = []
            for I in range(NCHUNK):
                for J in range(I + 1, NCHUNK):
                    tp = pstr.tile([P, P], bf16, tag="tr")
                    nc.tensor.transpose(
                        tp[:], Mt[:, blkoff(I, J) : blkoff(I, J) + P], identb[:]
                    )
                    tps_l.append((I, J, tp))
            # PSUM->SBUF copies of the transposed blocks: split ACT/DVE
            for k, (I, J, tp) in enumerate(tps_l):
                ab = atp.tile([P, P], bf16, tag="at")
                if k % 2 == 0:
                    nc.scalar.copy(ab[:], tp[:])
                else:
                    nc.vector.tensor_copy(ab[:], tp[:])
                at_sb[(I, J)] = ab

            poutQ = psout.tile([DIM, N], f32, tag="q")
            poutP = psout.tile([DIM, N], f32, tag="p")
            ncontrib = [0]
            NTOT = NCHUNK * NCHUNK  # 16 contributions per tile

            def contrib(row_chunk, stat_chunk, mov_ap):
                g = ncontrib[0]
                ncontrib[0] += 1
                cols = slice(row_chunk * P, (row_chunk + 1) * P)
                nc.tensor.matmul(
                    poutQ[:, cols],
                    statx6b[:, b, stat_chunk, 0:DIM],
                    mov_ap,
                    start=(g == 0),
                    stop=False,
                    skip_group_check=True,
                )
                nc.tensor.matmul(
                    poutP[:, cols],
                    statx6b[:, b, stat_chunk, DIM : 2 * DIM],
                    mov_ap,
                    start=(g == 0),
                    stop=(g == NTOT - 1),
                    skip_group_check=True,
                )

            for I in range(NCHUNK):
                contrib(I, I, Mt[:, blkoff(I, I) : blkoff(I, I) + P])
            for I in range(NCHUNK):
                for J in range(I + 1, NCHUNK):
                    contrib(J, I, Mt[:, blkoff(I, J) : blkoff(I, J) + P])
            for I in range(NCHUNK):
                for J in range(I + 1, NCHUNK):
                    contrib(I, J, at_sb[(I, J)][:])
            # mu fold into Q rows: Q' = Q - mu - c0'  (muAb = -c' replicated)
            hmu = hmup.tile([MU, N], bf16, tag="hmu")
            nc.scalar.activation(
                hmu[:],
                unrep_sb[:, b, :],
                AF.Exp,
                scale=negbeta_sb[:, 0:1],
                bias=eab_sb[0:MU, META : META + 1],
            )
            nc.tensor.matmul(
                poutQ[:, :],
                muAb[:],
                hmu[:],
                start=False,
                stop=True,
                skip_group_check=True,
            )
            return poutQ, poutP

        def emit_finalize(b, pq):
            poutQ, poutP = pq
            # out = (P' + c0*X_c) - x*(Q' + c0*N)
            o1 = finp.tile([DIM, N], f32, tag="o1")
            nc.vector.scalar_tensor_tensor(
                out=o1[:],
                in0=poutQ[:],
                scalar=c0 * float(N),
                in1=xb16[b][:],
                op0=OP.add,
                op1=OP.mult,
            )
            outrow = orp.tile([DIM, N], f32, tag="or")
            nc.vector.scalar_tensor_tensor(
                out=outrow[:],
                in0=poutP[:],
                scalar=c0x_sb[:, b : b + 1],
                in1=o1[:],
                op0=OP.add,
                op1=OP.subtract,
            )
            nc.gpsimd.dma_start(out=out_d[b], in_=outrow[:])

        # ---- schedule ----
        tps0 = emit_d2(0)
        acc0 = emit_expacc(0, tps0)
        tB0 = emit_tcopy(0, tps0)
        Mt0 = mpool.tile([P, NPACK], bf16, tag="m0")
        emit_horner(0, tB0, Mt0)
        tps1 = emit_d2(1)  # reuses the psd2 buffer once b0's reads are done
        acc1 = emit_expacc(1, tps1)
        tB1 = emit_tcopy(1, tps1)
        emit_merge(0, acc0, Mt0)
        pq0 = emit_contract(0, Mt0)
        Mt1 = mpool.tile([P, NPACK], bf16, tag="m1")
        emit_horner(1, tB1, Mt1)
        emit_merge(1, acc1, Mt1)
        emit_finalize(0, pq0)
        pq1 = emit_contract(1, Mt1)
        emit_finalize(1, pq1)

    _spread_sync_waits(nc)
    return nc


def _ensure_ntff_hook():
    """bass_utils' axon trace path imports antenv.axon_hooks, which the image's
    antenv package lacks. Register an equivalent module backed by the boot
    package's ctypes NTFF hook so trace=True works; degrade silently if the
    pieces are missing (tracing is optional)."""
    import os
    import types

    try:
        import antenv.axon_hooks  # noqa: F401

        return
    except ImportError:
        pass
    try:
        import antenv
    except ImportError:
        return
    mod = types.ModuleType("antenv.axon_hooks")
    box = {"h": None}
    mod.set_axon_ntff_profile_hook = lambda h: box.__setitem__("h", h)
    mod.get_axon_ntff_profile_hook = lambda: box["h"]
    sys.modules["antenv.axon_hooks"] = mod
    antenv.axon_hooks = mod
    try:
        from trn_agent_boot.trn_boot import _ntff_profile_via_ctypes

        so = "/opt/axon/libaxon_pjrt.so"
        if os.path.exists(so):
            hook = _ntff_profile_via_ctypes(so)
            if hook is not None:
                mod.set_axon_ntff_profile_hook(hook)
    except Exception:
        pass


def kernel(x, eta_w1, eta_b1, eta_w2, eta_b2, mu_w1, mu_b1, mu_w2, mu_b2):
    global LAST_RESULT
    _ensure_ntff_hook()
    import ml_dtypes
    from concourse.bass_utils import run_bass_kernel_spmd

    f32 = np.float32
    bf = ml_dtypes.bfloat16
    x = np.ascontiguousarray(np.asarray(x, dtype=f32))
    eta_w1 = np.asarray(eta_w1, f32)
    eta_b1 = np.asarray(eta_b1, f32)
    eta_w2 = np.asarray(eta_w2, f32)
    eta_b2 = np.asarray(eta_b2, f32)
    mu_w1 = np.asarray(mu_w1, f32)
    mu_b1 = np.asarray(mu_b1, f32)
    mu_w2 = np.asarray(mu_w2, f32)
    mu_b2 = np.asarray(mu_b2, f32)

    n2_all = (x.astype(np.float64) ** 2).sum(-1)  # [B, N]
    s, pc, eta_gam, eta_ce = _fit_surrogates(x, eta_w1, eta_b1, eta_w2, eta_b2)
    mu_g, mu_c = _fit_mu_exp(n2_all, mu_w1, mu_b1, mu_w2, mu_b2)
    c0 = float(pc[0])

    nc = _build_program(pc, eta_gam, eta_ce)

    DR = DIM + 2
    ident = np.eye(P, dtype=f32)
    identb = np.eye(P, dtype=f32).astype(bf)
    negbeta = (-mu_g[:, None]).astype(f32)  # [MU, 1]
    muAb = np.repeat(-mu_c[:, None], DIM, axis=1).astype(f32).astype(bf)
    ea_bias = eta_gam + np.log(np.abs(eta_ce))
    eab = np.zeros((P, META + 1), f32)
    eab[:, 0:META] = ea_bias[None, :].astype(f32)

    in_maps = []
    for core in range(NCORES):
        xc = x[core * BPC : (core + 1) * BPC]  # [BPC, N, DIM]
        xTc = xc.transpose(0, 2, 1)  # [BPC, DIM, N]
        n2 = n2_all[core * BPC : (core + 1) * BPC].astype(f32)  # [BPC, N]
        xTn = np.empty((DR, BPC, N), f32)
        xTn[0:DIM] = xTc.transpose(1, 0, 2)
        xTn[DIM] = n2
        xTn[DIM + 1] = 1.0
        statd = np.empty((DR, BPC, NCHUNK, P), f32)
        statx6 = np.empty((P, BPC, NCHUNK, 2 * DIM), f32)
        for bb in range(BPC):
            for I in range(NCHUNK):
                statd[0:DIM, bb, I, :] = -2.0 * s * xTc[bb, :, I * P : (I + 1) * P]
                statd[DIM, bb, I, :] = s
                statd[DIM + 1, bb, I, :] = s * n2[bb, I * P : (I + 1) * P] - 1.0
                statx6[:, bb, I, 0:DIM] = 1.0
                statx6[:, bb, I, DIM : 2 * DIM] = xc[bb, I * P : (I + 1) * P, :]
        unrep = np.broadcast_to(n2[None, :, :], (MU, BPC, N)).astype(f32)
        c0x = (c0 * xc.sum(axis=1).T).astype(f32)  # [DIM, BPC]
        in_maps.append(
            {
                "xTn": xTn,
                "statd": statd,
                "statx6": statx6.astype(bf),
                "ident": ident,
                "identb": identb,
                "unrep": np.ascontiguousarray(unrep),
                "negbeta": negbeta,
                "muAb": muAb,
                "c0x": c0x,
                "eab": eab,
            }
        )

    res = run_bass_kernel_spmd(nc, in_maps, core_ids=list(range(NCORES)))
    LAST_RESULT = res
    out = np.concatenate([r["out"] for r in res.results], axis=0)  # [B, DIM, N]
    return np.ascontiguousarray(out.transpose(0, 2, 1)).astype(np.float32)


# revision 35
# speedup vs baseline: 1.2936x; 1.2936x over previous
"""Trainium2 Bass kernel for the Backflow module.

Math (B=16, N=512, DIM=3, H=32):
  out[b,i,:] = sum_j eta(||x_bi - x_bj||) * (x_bi - x_bj)  +  mu(||x_bi||) * x_bi
where eta/mu are 1->H->1 tanh MLPs. The reference's eye()/diagonal correction
cancels exactly (eta(0)*(x_i - x_i) = 0 in the matrix form below).

Sharding: data-parallel over batch, 2 batches per core on 8 cores.

eta and mu are univariate scalar functions and the rel-err budget (2e-2)
is large, so we fit cheap surrogates at call time from the actual weights,
both in u = d^2 (no sqrt anywhere; exp/identity/copy live in one ACT
table set -> a single table load):

  t[i,j] = 2*d_ij^2/umax - 1 comes straight out of the PE: the d^2
  matmul carries two extra rows ([-2sx | s | s*n2_i - 1] stationary x
  [x | n2_j | 1] moving, fp16 = 1 cyc/row) so PSUM holds t directly;
  ACT copies the A-columns and DVE the B-columns to SBUF.

  M[i,j] := -eta(d_ij) - c0 evaluated two ways on disjoint column regions
  of the packed strip:
   A-region (ACT+PE): sum_m c_m exp(g_m (t+1)) - META exp ACT passes,
     |c_m| folded into the bias, sign via +/-identity fp16 stationaries
     accumulated on the PE into PSUM; one copy -> bf16 M tile.
   B-region (DVE): monomial Horner for P(t) - c0 via stock
     scalar_tensor_tensor ops (g = c_deg*t; g = (g + c_j)*t).
  The split ratio load-balances ACT vs DVE.

  The shared constant c0 is folded into the finalize for free:
  out_c[j] = (P'_c[j] + c0*X_c) - x_c[j]*(Q'[j] + c0*N), X_c = sum_i x_c[i].

  mu(||x_i||) = c0' + sum_m c'_m exp(-b_m n_i^2): ONE ACT exp pass on a
  [MU, N] broadcast of n^2 (per-partition scale), folded into the Q rows
  of the PSUM contraction with a negated bf16 stationary.

Per-core layout: i on partitions (4 chunks of 128), j on the free dim.
Symmetry eta(d_ij) = eta(d_ji): compute only block-triangular strips
(chunk I covers j in [128*I, 512)), packed to [128, 1280] with
bank-aligned chunk offsets (order 0,1,3,2) so every matmul output stays
inside a PSUM bank.

Row sums via one merged PE contraction per block: 6-wide stationary
[1,1,1 | x_c] in bf16 against the bf16 M block (1 cyc/row) into a
[6, N] PSUM tile (Q rows first so the mu fold lands at base partition
0). Direct blocks give the (J,*) rows; the 6 PE-transposed blocks land
in ONE PSUM bank and return to SBUF in a single DVE copy per batch.
"""

import sys

sys.path.insert(0, "/opt/trn_rl_repo")

import numpy as np
from contextlib import ExitStack

B, N, DIM, H = 16, 512, 3, 32
NCORES = 8
BPC = B // NCORES  # batches per core
P = 128
NCHUNK = N // P  # 4
WIDTHS = [N - P * I for I in range(NCHUNK)]  # [512, 384, 256, 128]
# bank-aligned packing of the block-triangular strips (chunk order 0,1,3,2):
# every chunk's [128, W] matmul output stays inside 2KB PSUM banks.
OFFS = [0, 512, 1024, 896]
NPACK = sum(WIDTHS)  # 1280

DEG = 11  # B-region polynomial degree
META = 6  # A-region exp basis size
MU = 12  # mu exp-basis size (incl. the g=0 constant term)
ASPLIT = 768  # packed columns [0, ASPLIT) on ACT path, rest on DVE path
ASEGS = [(0, 512), (512, 256)]  # accumulate matmul splits (PSUM banks, >=256)
assert ASEGS[-1][0] + ASEGS[-1][1] == ASPLIT

LAST_RESULT = None


def _spread_sync_waits(nc):
    """The pinned walrus rejects instructions carrying more than one sync wait
    ('Too many sync wait commands'). Engines execute their instruction streams
    in order, so hoist all-but-one wait of any such instruction onto same-engine
    NoOps inserted directly before it — semantically identical ordering."""
    from concourse import mybir

    n_added = 0
    for bb in nc.main_func.blocks:
        insts = bb.instructions
        i = 0
        while i < len(insts):
            inst = insts[i]
            si = getattr(inst, "sync_info", None)
            waits = list(si.on_wait) if si is not None and si.on_wait else []
            if len(waits) > 1:
                si.on_wait = waits[-1:]
                for k, w in enumerate(waits[:-1]):
                    nop = mybir.InstNoOp(
                        name=f"{inst.name}-wspread{k}",
                        sync_info=mybir.SyncInfo(on_wait=[w], on_update=[]),
                        engine=inst.engine,
                        bass_nofuse=True,
                    )
                    insts.insert(i + k, nop)
                    n_added += 1
                i += len(waits) - 1
            i += 1
    return n_added


def _eta_fn(d, w1, b1, w2, b2):
    return np.tanh(d[..., None] * w1[0] + b1) @ w2[:, 0] + b2[0]


def _fit_surrogates(x, eta_w1, eta_b1, eta_w2, eta_b2):
    """Global fits of f(t) = -eta(sqrt(u)), t = 2u/umax - 1:
    poly (ascending monomial coeffs, deg DEG) and exp basis
    f - c0 ~= sum_m c_m exp(g_m (t+1)). Returns (s, pc, gam, ce)."""
    x = x.astype(np.float64)
    n2 = (x**2).sum(-1)  # [B, N]
    rng = np.random.default_rng(0)
    umax = 0.0
    samples = []
    for b in range(B):
        G = x[b] @ x[b].T
        Ub = np.maximum(n2[b][:, None] + n2[b][None, :] - 2 * G, 0.0)
        umax = max(umax, float(Ub.max()))
        idx = rng.choice(N * N, 16384, replace=False)
        samples.append(Ub.reshape(-1)[idx])
    umax = umax * 1.002 + 1e-6
    uu = np.concatenate(samples)
    ug = np.linspace(0.0, umax, 2000)
    ufit = np.concatenate([uu, ug])
    w = np.concatenate(
        [np.sqrt(np.sqrt(uu) + 0.1), 3.0 * np.sqrt(np.sqrt(ug) + 0.1)]
    )
    tfit = 2.0 * ufit / umax - 1.0
    yfit = -_eta_fn(np.sqrt(ufit), eta_w1, eta_b1, eta_w2, eta_b2)
    import numpy.polynomial.chebyshev as Ch

    cf = Ch.chebfit(tfit, yfit, DEG, w=w)
    pc = Ch.cheb2poly(cf)  # ascending monomial coeffs in t
    c0 = float(pc[0])
    # exp basis on the residual target f - c0, no free constant
    gam = -np.geomspace(0.08, 48.0, META)  # exponents per (t+1) unit
    A = np.exp((tfit[:, None] + 1.0) * gam[None, :])
    Aw = A * w[:, None]
    ce, *_ = np.linalg.lstsq(Aw, (yfit - c0) * w, rcond=None)
    s = 2.0 / umax
    return float(s), pc.astype(np.float64), gam, ce


def _fit_mu_exp(n2_all, mu_w1, mu_b1, mu_w2, mu_b2):
    """Fit mu(sqrt(u)) ~= sum_m c_m exp(-g_m u) on the actual n^2 values
    (the exact evaluation points). g_0 = 0 supplies the constant term."""
    us = np.sort(n2_all.reshape(-1).astype(np.float64))
    n2max = float(us[-1]) * 1.001 + 1e-9
    g = np.concatenate([[0.0], np.geomspace(0.125, 96.0, MU - 1) / n2max])
    A = np.exp(-us[:, None] * g[None, :])
    y = _eta_fn(np.sqrt(us), mu_w1, mu_b1, mu_w2, mu_b2)
    w = np.sqrt(np.sqrt(us) + 0.1)
    Aw = A * w[:, None]
    AtA = Aw.T @ Aw + 1e-10 * len(us) * np.eye(MU)
    c = np.linalg.solve(AtA, Aw.T @ (y * w))
    return g.astype(np.float64), c.astype(np.float64)


def _build_program(poly_pc, eta_gam, eta_ce):
    import concourse.bass as bass
    import concourse.tile as tile
    from concourse import mybir

    f32 = mybir.dt.float32
    f16 = mybir.dt.float16
    f32r = mybir.dt.float32r
    bf16 = mybir.dt.bfloat16
    AF = mybir.ActivationFunctionType
    OP = mybir.AluOpType

    pc = [float(v) for v in poly_pc]  # ascending, len DEG+1
    c0 = pc[0]
    ea_scale = [float(g) for g in eta_gam]
    ea_sign = [1.0 if c > 0 else -1.0 for c in eta_ce]

    DR = DIM + 2  # d^2 matmul rows: x(3), n2, ones

    nc = bass.Bass()
    xTn_d = nc.dram_tensor("xTn", [DR, BPC, N], f16, kind="ExternalInput")
    statd_d = nc.dram_tensor("statd", [DR, BPC, NCHUNK, P], f16, kind="ExternalInput")
    statx6_d = nc.dram_tensor("statx6", [P, BPC, NCHUNK, 2 * DIM], bf16, kind="ExternalInput")
    identb_d = nc.dram_tensor("identb", [P, P], bf16, kind="ExternalInput")
    identh_d = nc.dram_tensor("identh", [P, 2, P], f16, kind="ExternalInput")
    unrep_d = nc.dram_tensor("unrep", [MU, BPC, N], f32, kind="ExternalInput")
    negbeta_d = nc.dram_tensor("negbeta", [MU, 1], f32, kind="ExternalInput")
    muAb_d = nc.dram_tensor("muAb", [MU, DIM], bf16, kind="ExternalInput")
    c0x_d = nc.dram_tensor("c0x", [DIM, BPC], f32, kind="ExternalInput")
    eab_d = nc.dram_tensor("eab", [P, META + 1], f32, kind="ExternalInput")
    xb_d = nc.dram_tensor("xb", [DIM, BPC, N], bf16, kind="ExternalInput")
    out_d = nc.dram_tensor("out", [BPC, DIM, N], f32, kind="ExternalOutput")

    with tile.TileContext(nc) as tc, ExitStack() as ctx:
        singles = ctx.enter_context(tc.tile_pool(name="singles", bufs=1))
        tap = ctx.enter_context(tc.tile_pool(name="tap", bufs=2))
        tbp = ctx.enter_context(tc.tile_pool(name="tbp", bufs=2))
        hpool = ctx.enter_context(tc.tile_pool(name="hpool", bufs=2))
        hsp = ctx.enter_context(tc.tile_pool(name="hsp", bufs=8))
        mpool = ctx.enter_context(tc.tile_pool(name="mpool", bufs=2))
        atp = ctx.enter_context(tc.tile_pool(name="atp", bufs=2))
        hmup = ctx.enter_context(tc.tile_pool(name="hmup", bufs=2))
        pqp = ctx.enter_context(tc.tile_pool(name="pqp", bufs=2))
        finp = ctx.enter_context(tc.tile_pool(name="finp", bufs=2))
        orp = ctx.enter_context(tc.tile_pool(name="orp", bufs=2))
        psd2 = ctx.enter_context(tc.tile_pool(name="psd2", bufs=1, space="PSUM"))
        psacc = ctx.enter_context(tc.tile_pool(name="psacc", bufs=1, space="PSUM"))
        psout = ctx.enter_context(tc.tile_pool(name="psout", bufs=1, space="PSUM"))
        pstr = ctx.enter_context(tc.tile_pool(name="pstr", bufs=1, space="PSUM"))

        # ---- inputs; d^2-path tensors first (they gate the first matmul) ----
        statd_sb = singles.tile([DR, BPC, NCHUNK, P], f16)
        nc.gpsimd.dma_start(out=statd_sb[:], in_=statd_d[:])
        xTn_sb = singles.tile([DR, BPC, N], f16)
        nc.gpsimd.dma_start(out=xTn_sb[:], in_=xTn_d[:])
        eab_sb = singles.tile([P, META + 1], f32)
        nc.gpsimd.dma_start(out=eab_sb[:], in_=eab_d[:])
        identh = singles.tile([P, 2, P], f16)  # [:,0,:]=+I, [:,1,:]=-I
        nc.gpsimd.dma_start(out=identh[:], in_=identh_d[:])
        statx6b = singles.tile([P, BPC, NCHUNK, 2 * DIM], bf16)
        nc.gpsimd.dma_start(out=statx6b[:], in_=statx6_d[:])
        identb = singles.tile([P, P], bf16)
        nc.gpsimd.dma_start(out=identb[:], in_=identb_d[:])
        xb_sb = singles.tile([DIM, BPC, N], bf16)
        nc.gpsimd.dma_start(out=xb_sb[:], in_=xb_d[:])
        unrep_sb = singles.tile([MU, BPC, N], f32)
        nc.gpsimd.dma_start(out=unrep_sb[:], in_=unrep_d[:])
        negbeta_sb = singles.tile([MU, 1], f32)
        nc.gpsimd.dma_start(out=negbeta_sb[:], in_=negbeta_d[:])
        muAb = singles.tile([MU, DIM], bf16)
        nc.gpsimd.dma_start(out=muAb[:], in_=muAb_d[:])
        c0x_sb = singles.tile([DIM, BPC], f32)
        nc.gpsimd.dma_start(out=c0x_sb[:], in_=c0x_d[:])

        # ---- t strips straight from the PE (fp16 operands, 1 cyc/row) ----
        def emit_d2(b):
            tps = psd2.tile([P, NPACK], f32, tag="t")
            for I in range(NCHUNK):
                nc.tensor.matmul(
                    tps[:, OFFS[I] : OFFS[I] + WIDTHS[I]],
                    statd_sb[:, b, I, :],
                    xTn_sb[:, b, P * I : N],
                    start=True,
                    stop=True,
                    skip_group_check=True,
                )
            return tps

        def emit_tcopies(b, tps):
            """PSUM t -> SBUF: ACT takes the A columns, DVE the B columns."""
            tA = tap.tile([P, ASPLIT], f32, tag="ta")
            nc.scalar.copy(tA[:], tps[:, 0:ASPLIT])
            tB = tbp.tile([P, NPACK - ASPLIT], f32, tag="tb")
            nc.vector.tensor_copy(tB[:], tps[:, ASPLIT:NPACK])
            return tA, tB

        def emit_expacc(b, tA):
            """A-region: META exp passes on ACT, +/-I fp16 accumulate on PE."""
            acc = psacc.tile([P, ASPLIT], f32, tag="acc")
            for m in range(META):
                hs = hsp.tile([P, ASPLIT], f16, tag="hs")
                nc.scalar.activation(
                    hs[:],
                    tA[:],
                    AF.Exp,
                    scale=ea_scale[m],
                    bias=eab_sb[:, m : m + 1],
                )
                sgn = 0 if ea_sign[m] > 0 else 1
                for off, w in ASEGS:
                    nc.tensor.matmul(
                        acc[:, off : off + w],
                        identh[:, sgn, :],
                        hs[:, off : off + w],
                        start=(m == 0),
                        stop=(m == META - 1),
                        skip_group_check=True,
                    )
            return acc

        def emit_horner(b, tB, Mt):
            """B-region: monomial Horner for P(t) - c0 on DVE (stock ops)."""
            g = hpool.tile([P, NPACK - ASPLIT], f32, tag="h")
            nc.vector.tensor_scalar_mul(out=g[:], in0=tB[:], scalar1=pc[DEG])
            gap = g[:]
            for j in range(DEG - 1, 0, -1):
                if j == 1:
                    dst_ap = Mt[:, ASPLIT:NPACK]
                else:
                    dst = hpool.tile([P, NPACK - ASPLIT], f32, tag="h")
                    dst_ap = dst[:]
                nc.vector.scalar_tensor_tensor(
                    out=dst_ap,
                    in0=gap,
                    scalar=pc[j],
                    in1=tB[:],
                    op0=OP.add,
                    op1=OP.mult,
                )
                gap = dst_ap

        def emit_merge(b, acc, Mt):
            nc.scalar.copy(Mt[:, 0:ASPLIT], acc[:])

        def blkoff(I, J):
            return OFFS[I] + (J - I) * P

        PAIRS = [(I, J) for I in range(NCHUNK) for J in range(I + 1, NCHUNK)]

        def emit_transposes(b, Mt):
            # all 6 transposed blocks into ONE PSUM bank, one DVE copy back
            tp = pstr.tile([P, len(PAIRS), P], bf16, tag="tr")
            for k, (I, J) in enumerate(PAIRS):
                nc.tensor.transpose(
                    tp[:, k, :], Mt[:, blkoff(I, J) : blkoff(I, J) + P], identb[:]
                )
            at = atp.tile([P, len(PAIRS), P], bf16, tag="at")
            nc.vector.tensor_copy(at[:], tp[:])
            return at

        def emit_contract(b, Mt, at):
            poutQ = psout.tile([DIM, N], f32, tag="q")
            poutP = psout.tile([DIM, N], f32, tag="p")
            ncontrib = [0]
            NTOT = NCHUNK * NCHUNK  # 16 contributions per tile

            def contrib(row_chunk, stat_chunk, mov_ap):
                g = ncontrib[0]
                ncontrib[0] += 1
                cols = slice(row_chunk * P, (row_chunk + 1) * P)
                nc.tensor.matmul(
                    poutQ[:, cols],
                    statx6b[:, b, stat_chunk, 0:DIM],
                    mov_ap,
                    start=(g == 0),
                    stop=False,
                    skip_group_check=True,
                )
                nc.tensor.matmul(
                    poutP[:, cols],
                    statx6b[:, b, stat_chunk, DIM : 2 * DIM],
                    mov_ap,
                    start=(g == 0),
                    stop=(g == NTOT - 1),
                    skip_group_check=True,
                )

            for I in range(NCHUNK):
                contrib(I, I, Mt[:, blkoff(I, I) : blkoff(I, I) + P])
            for I, J in PAIRS:
                contrib(J, I, Mt[:, blkoff(I, J) : blkoff(I, J) + P])
            for k, (I, J) in enumerate(PAIRS):
                contrib(I, J, at[:, k, :])
            # mu fold into Q rows: Q' = Q - mu - c0'  (muAb = -c' replicated)
            hmu = hmup.tile([MU, N], bf16, tag="hmu")
            nc.scalar.activation(
                hmu[:],
                unrep_sb[:, b, :],
                AF.Exp,
                scale=negbeta_sb[:, 0:1],
                bias=eab_sb[0:MU, META : META + 1],
            )
            nc.tensor.matmul(
                poutQ[:, :],
                muAb[:],
                hmu[:],
                start=False,
                stop=True,
                skip_group_check=True,
            )
            return poutQ, poutP

        def emit_finalize(b, pq):
            poutQ, poutP = pq
            # out = (P' + c0*X_c) - x*(Q' + c0*N)
            o1 = finp.tile([DIM, N], f32, tag="o1")
            nc.vector.scalar_tensor_tensor(
                out=o1[:],
                in0=poutQ[:],
                scalar=c0 * float(N),
                in1=xb_sb[:, b, :],
                op0=OP.add,
                op1=OP.mult,
            )
            outrow = orp.tile([DIM, N], f32, tag="or")
            nc.vector.scalar_tensor_tensor(
                out=outrow[:],
                in0=poutP[:],
                scalar=c0x_sb[:, b : b + 1],
                in1=o1[:],
                op0=OP.add,
                op1=OP.subtract,
            )
            nc.gpsimd.dma_start(out=out_d[b], in_=outrow[:])

        # ---- schedule ----
        tps0 = emit_d2(0)
        tA0, tB0 = emit_tcopies(0, tps0)
        tps1 = emit_d2(1)  # reuses the psd2 buffer once b0's copies are done
        acc0 = emit_expacc(0, tA0)
        Mt0 = mpool.tile([P, NPACK], bf16, tag="m0")
        emit_horner(0, tB0, Mt0)
        tA1, tB1 = emit_tcopies(1, tps1)
        acc1 = emit_expacc(1, tA1)
        emit_merge(0, acc0, Mt0)
        at0 = emit_transposes(0, Mt0)
        Mt1 = mpool.tile([P, NPACK], bf16, tag="m1")
        emit_horner(1, tB1, Mt1)
        pq0 = emit_contract(0, Mt0, at0)
        emit_merge(1, acc1, Mt1)
        at1 = emit_transposes(1, Mt1)
        emit_finalize(0, pq0)
        pq1 = emit_contract(1, Mt1, at1)
        emit_finalize(1, pq1)

    _spread_sync_waits(nc)
    return nc


def _ensure_ntff_hook():
    """bass_utils' axon trace path imports antenv.axon_hooks, which the image's
    antenv package lacks. Register an equivalent module backed by the boot
    package's ctypes NTFF hook so trace=True works; degrade silently if the
    pieces are missing (tracing is optional)."""
    import os
    import types

    try:
        import antenv.axon_hooks  # noqa: F401

        return
    except ImportError:
        pass
    try:
        import antenv
    except ImportError:
        return
    mod = types.ModuleType("antenv.axon_hooks")
    box = {"h": None}
    mod.set_axon_ntff_profile_hook = lambda h: box.__setitem__("h", h)
    mod.get_axon_ntff_profile_hook = lambda: box["h"]
    sys.modules["antenv.axon_hooks"] = mod
    antenv.axon_hooks = mod
    try:
        from trn_agent_boot.trn_boot import _ntff_profile_via_ctypes

        so = "/opt/axon/libaxon_pjrt.so"
        if os.path.exists(so):
            hook = _ntff_profile_via_ctypes(so)
            if hook is not None:
                mod.set_axon_ntff_profile_hook(hook)
    except Exception:
        pass


def kernel(x, eta_w1, eta_b1, eta_w2, eta_b2, mu_w1, mu_b1, mu_w2, mu_b2):
    global LAST_RESULT
    _ensure_ntff_hook()
    import ml_dtypes
    from concourse.bass_utils import run_bass_kernel_spmd

    f32 = np.float32
    f16 = np.float16
    bf = ml_dtypes.bfloat16
    x = np.ascontiguousarray(np.asarray(x, dtype=f32))
    eta_w1 = np.asarray(eta_w1, f32)
    eta_b1 = np.asarray(eta_b1, f32)
    eta_w2 = np.asarray(eta_w2, f32)
    eta_b2 = np.asarray(eta_b2, f32)
    mu_w1 = np.asarray(mu_w1, f32)
    mu_b1 = np.asarray(mu_b1, f32)
    mu_w2 = np.asarray(mu_w2, f32)
    mu_b2 = np.asarray(mu_b2, f32)

    n2_all = (x.astype(np.float64) ** 2).sum(-1)  # [B, N]
    s, pc, eta_gam, eta_ce = _fit_surrogates(x, eta_w1, eta_b1, eta_w2, eta_b2)
    mu_g, mu_c = _fit_mu_exp(n2_all, mu_w1, mu_b1, mu_w2, mu_b2)
    c0 = float(pc[0])

    nc = _build_program(pc, eta_gam, eta_ce)

    DR = DIM + 2
    identb = np.eye(P, dtype=f32).astype(bf)
    identh = np.empty((P, 2, P), f16)
    identh[:, 0, :] = np.eye(P, dtype=f32)
    identh[:, 1, :] = -np.eye(P, dtype=f32)
    negbeta = (-mu_g[:, None]).astype(f32)  # [MU, 1]
    muAb = np.repeat(-mu_c[:, None], DIM, axis=1).astype(f32).astype(bf)
    ea_bias = eta_gam + np.log(np.abs(eta_ce))
    eab = np.zeros((P, META + 1), f32)
    eab[:, 0:META] = ea_bias[None, :].astype(f32)

    in_maps = []
    for core in range(NCORES):
        xc = x[core * BPC : (core + 1) * BPC]  # [BPC, N, DIM]
        xTc = xc.transpose(0, 2, 1)  # [BPC, DIM, N]
        n2 = n2_all[core * BPC : (core + 1) * BPC].astype(f32)  # [BPC, N]
        xTn = np.empty((DR, BPC, N), f32)
        xTn[0:DIM] = xTc.transpose(1, 0, 2)
        xTn[DIM] = n2
        xTn[DIM + 1] = 1.0
        statd = np.empty((DR, BPC, NCHUNK, P), f32)
        statx6 = np.empty((P, BPC, NCHUNK, 2 * DIM), f32)
        for bb in range(BPC):
            for I in range(NCHUNK):
                statd[0:DIM, bb, I, :] = -2.0 * s * xTc[bb, :, I * P : (I + 1) * P]
                statd[DIM, bb, I, :] = s
                statd[DIM + 1, bb, I, :] = s * n2[bb, I * P : (I + 1) * P] - 1.0
                statx6[:, bb, I, 0:DIM] = 1.0
                statx6[:, bb, I, DIM : 2 * DIM] = xc[bb, I * P : (I + 1) * P, :]
        unrep = np.broadcast_to(n2[None, :, :], (MU, BPC, N)).astype(f32)
        c0x = (c0 * xc.sum(axis=1).T).astype(f32)  # [DIM, BPC]
        in_maps.append(
            {
                "xTn": xTn.astype(f16),
                "statd": statd.astype(f16),
                "statx6": statx6.astype(bf),
                "identb": identb,
                "identh": identh,
                "unrep": np.ascontiguousarray(unrep),
                "negbeta": negbeta,
                "muAb": muAb,
                "c0x": c0x,
                "eab": eab,
                "xb": np.ascontiguousarray(xTc.transpose(1, 0, 2)).astype(bf),
            }
        )

    res = run_bass_kernel_spmd(nc, in_maps, core_ids=list(range(NCORES)))
    LAST_RESULT = res
    out = np.concatenate([r["out"] for r in res.results], axis=0)  # [B, DIM, N]
    return np.ascontiguousarray(out.transpose(0, 2, 1)).astype(np.float32)


# revision 38
# speedup vs baseline: 1.5340x; 1.1858x over previous
"""Trainium2 Bass kernel for the Backflow module.

Math (B=16, N=512, DIM=3, H=32):
  out[b,i,:] = sum_j eta(||x_bi - x_bj||) * (x_bi - x_bj)  +  mu(||x_bi||) * x_bi
where eta/mu are 1->H->1 tanh MLPs. The reference's eye()/diagonal correction
cancels exactly (eta(0)*(x_i - x_i) = 0 in the matrix form below).

Sharding: data-parallel over batch, 2 batches per core on 8 cores.

eta and mu are univariate scalar functions and the rel-err budget (2e-2)
is large, so we fit cheap surrogates at call time from the actual weights,
both in u = d^2 (no sqrt anywhere; exp/identity/copy live in one ACT
table set -> a single table load):

  t[i,j] = 2*d_ij^2/umax - 1 comes straight out of the PE: the d^2
  matmul carries two extra rows ([-2sx | s | s*n2_i - 1] stationary x
  [x | n2_j | 1] moving, fp16 = 1 cyc/row) so PSUM holds t directly;
  ACT copies the A-columns and DVE the B-columns to SBUF.

  M[i,j] := -eta(d_ij) - c0 evaluated two ways on disjoint column regions
  of the packed strip:
   A-region (ACT+PE): sum_m c_m exp(g_m (t+1)) - META exp ACT passes,
     |c_m| folded into the bias, sign via +/-identity fp16 stationaries
     accumulated on the PE into PSUM; one ACT copy -> bf16 M tile.
   B-region (DVE): monomial Horner for P(t) - c0 via stock
     scalar_tensor_tensor ops (g = c_deg*t; g = (g + c_j)*t).
  The split ratio load-balances ACT vs DVE.

  The shared constant c0 is folded into the finalize for free:
  out_c[j] = (P'_c[j] + c0*X_c) - x_c[j]*(Q'[j] + c0*N), X_c = sum_i x_c[i].

  mu(||x_i||) = c0' + sum_m c'_m exp(-b_m n_i^2): ONE ACT exp pass on a
  [MU, N] broadcast of n^2 (per-partition scale), folded into the Q rows
  of the PSUM contraction with a negated bf16 stationary.

Per-core layout: i on partitions (4 chunks of 128), j on the free dim.
Symmetry eta(d_ij) = eta(d_ji): compute only block-triangular strips
(chunk I covers j in [128*I, 512)), packed to [128, 1280] with
bank-aligned chunk offsets (order 0,1,3,2) so every matmul output stays
inside a PSUM bank.

Row sums via PE contractions (3-wide ones / x stationaries in bf16, M
moving in bf16 = 1 cyc/row), merged per stationary chunk: the direct
contributions of stationary chunk I cover the contiguous strip
[OFFS[I], OFFS[I]+W) -> one matmul per (I, P/Q); the reflected blocks
(via 6 PE transposes into ONE PSUM bank, one DVE copy back, ordered by
J) also merge per stationary J. Interleaving b1's d^2 matmuls into
b0's accumulate gaps keeps the PE p-state up. Input DMAs are packed
into few tensors and triggered from different engines' queues.
"""

import sys

sys.path.insert(0, "/opt/trn_rl_repo")

import numpy as np
from contextlib import ExitStack

B, N, DIM, H = 16, 512, 3, 32
NCORES = 8
BPC = B // NCORES  # batches per core
P = 128
NCHUNK = N // P  # 4
WIDTHS = [N - P * I for I in range(NCHUNK)]  # [512, 384, 256, 128]
# bank-aligned packing of the block-triangular strips (chunk order 0,1,3,2)
OFFS = [0, 512, 1024, 896]
NPACK = sum(WIDTHS)  # 1280

DEG = 8  # B-region polynomial degree
META = 6  # A-region exp basis size
MU = 12  # mu exp-basis size (incl. the g=0 constant term)
ASPLIT = 768  # packed columns [0, ASPLIT) on ACT path, rest on DVE path
ASEGS = [(0, 512), (512, 256)]  # accumulate matmul splits (PSUM banks, >=256)
assert ASEGS[-1][0] + ASEGS[-1][1] == ASPLIT

# transposed-block pairs ordered by stationary chunk J, then I;
# J-group g starts at index J*(J-1)/2 and holds J blocks (I = 0..J-1)
PAIRS_BYJ = [(I, J) for J in range(1, NCHUNK) for I in range(J)]

LAST_RESULT = None


def _spread_sync_waits(nc):
    """The pinned walrus rejects instructions carrying more than one sync wait
    ('Too many sync wait commands'). Engines execute their instruction streams
    in order, so hoist all-but-one wait of any such instruction onto same-engine
    NoOps inserted directly before it — semantically identical ordering."""
    from concourse import mybir

    n_added = 0
    for bb in nc.main_func.blocks:
        insts = bb.instructions
        i = 0
        while i < len(insts):
            inst = insts[i]
            si = getattr(inst, "sync_info", None)
            waits = list(si.on_wait) if si is not None and si.on_wait else []
            if len(waits) > 1:
                si.on_wait = waits[-1:]
                for k, w in enumerate(waits[:-1]):
                    nop = mybir.InstNoOp(
                        name=f"{inst.name}-wspread{k}",
                        sync_info=mybir.SyncInfo(on_wait=[w], on_update=[]),
                        engine=inst.engine,
                        bass_nofuse=True,
                    )
                    insts.insert(i + k, nop)
                    n_added += 1
                i += len(waits) - 1
            i += 1
    return n_added


def _eta_fn(d, w1, b1, w2, b2):
    return np.tanh(d[..., None] * w1[0] + b1) @ w2[:, 0] + b2[0]


def _fit_surrogates(x, eta_w1, eta_b1, eta_w2, eta_b2):
    """Global fits of f(t) = -eta(sqrt(u)), t = 2u/umax - 1:
    poly (ascending monomial coeffs, deg DEG) and exp basis
    f - c0 ~= sum_m c_m exp(g_m (t+1)). Returns (s, pc, gam, ce)."""
    x = x.astype(np.float64)
    n2 = (x**2).sum(-1)  # [B, N]
    rng = np.random.default_rng(0)
    umax = 0.0
    samples = []
    for b in range(B):
        G = x[b] @ x[b].T
        Ub = np.maximum(n2[b][:, None] + n2[b][None, :] - 2 * G, 0.0)
        umax = max(umax, float(Ub.max()))
        idx = rng.choice(N * N, 16384, replace=False)
        samples.append(Ub.reshape(-1)[idx])
    umax = umax * 1.002 + 1e-6
    uu = np.concatenate(samples)
    ug = np.linspace(0.0, umax, 2000)
    ufit = np.concatenate([uu, ug])
    w = np.concatenate(
        [np.sqrt(np.sqrt(uu) + 0.1), 3.0 * np.sqrt(np.sqrt(ug) + 0.1)]
    )
    tfit = 2.0 * ufit / umax - 1.0
    yfit = -_eta_fn(np.sqrt(ufit), eta_w1, eta_b1, eta_w2, eta_b2)
    import numpy.polynomial.chebyshev as Ch

    cf = Ch.chebfit(tfit, yfit, DEG, w=w)
    pc = Ch.cheb2poly(cf)  # ascending monomial coeffs in t
    c0 = float(pc[0])
    # exp basis on the residual target f - c0, no free constant
    gam = -np.geomspace(0.08, 48.0, META)  # exponents per (t+1) unit
    A = np.exp((tfit[:, None] + 1.0) * gam[None, :])
    Aw = A * w[:, None]
    ce, *_ = np.linalg.lstsq(Aw, (yfit - c0) * w, rcond=None)
    s = 2.0 / umax
    return float(s), pc.astype(np.float64), gam, ce


def _fit_mu_exp(n2_all, mu_w1, mu_b1, mu_w2, mu_b2):
    """Fit mu(sqrt(u)) ~= sum_m c_m exp(-g_m u) on the actual n^2 values
    (the exact evaluation points). g_0 = 0 supplies the constant term."""
    us = np.sort(n2_all.reshape(-1).astype(np.float64))
    n2max = float(us[-1]) * 1.001 + 1e-9
    g = np.concatenate([[0.0], np.geomspace(0.125, 96.0, MU - 1) / n2max])
    A = np.exp(-us[:, None] * g[None, :])
    y = _eta_fn(np.sqrt(us), mu_w1, mu_b1, mu_w2, mu_b2)
    w = np.sqrt(np.sqrt(us) + 0.1)
    Aw = A * w[:, None]
    AtA = Aw.T @ Aw + 1e-10 * len(us) * np.eye(MU)
    c = np.linalg.solve(AtA, Aw.T @ (y * w))
    return g.astype(np.float64), c.astype(np.float64)


# packed f32 smalls blob layout: [P, FPK] with
#   cols [0, META+1): eab (exp-basis biases + mu zero bias col)
#   col META+1: negbeta (rows 0:MU)
#   cols META+2 .. META+3: c0x (rows 0:DIM)
FPK = META + 2 + BPC


def _build_program(poly_pc, eta_gam, eta_ce):
    import concourse.bass as bass
    import concourse.tile as tile
    from concourse import mybir

    f32 = mybir.dt.float32
    f16 = mybir.dt.float16
    bf16 = mybir.dt.bfloat16
    AF = mybir.ActivationFunctionType
    OP = mybir.AluOpType

    pc = [float(v) for v in poly_pc]  # ascending, len DEG+1
    c0 = pc[0]
    ea_scale = [float(g) for g in eta_gam]
    ea_sign = [1.0 if c > 0 else -1.0 for c in eta_ce]

    DR = DIM + 2  # d^2 matmul rows: x(3), n2, ones
    NPAIR = len(PAIRS_BYJ)

    nc = bass.Bass()
    # dpack: statd [DR, BPC*NCHUNK*P] | xTn [DR, BPC*N]  (fp16, critical)
    dpack_d = nc.dram_tensor("dpack", [DR, BPC * NCHUNK * P + BPC * N], f16, kind="ExternalInput")
    identh_d = nc.dram_tensor("identh", [P, 2, P], f16, kind="ExternalInput")
    # bpack: statx6 (BPC*NCHUNK*6 cols) | identb (P cols), rows 0:MU of the
    # first 3 cols after that hold muAb
    BPK = BPC * NCHUNK * 2 * DIM + P + DIM
    bpack_d = nc.dram_tensor("bpack", [P, BPK], bf16, kind="ExternalInput")
    xb_d = nc.dram_tensor("xb", [DIM, BPC, N], bf16, kind="ExternalInput")
    unrep_d = nc.dram_tensor("unrep", [MU, BPC, N], f16, kind="ExternalInput")
    fpack_d = nc.dram_tensor("fpack", [P, FPK], f32, kind="ExternalInput")
    out_d = nc.dram_tensor("out", [BPC, DIM, N], f32, kind="ExternalOutput")

    with tile.TileContext(nc) as tc, ExitStack() as ctx:
        singles = ctx.enter_context(tc.tile_pool(name="singles", bufs=1))
        tap = ctx.enter_context(tc.tile_pool(name="tap", bufs=2))
        tbp = ctx.enter_context(tc.tile_pool(name="tbp", bufs=2))
        hpool = ctx.enter_context(tc.tile_pool(name="hpool", bufs=2))
        hsp = ctx.enter_context(tc.tile_pool(name="hsp", bufs=8))
        mpool = ctx.enter_context(tc.tile_pool(name="mpool", bufs=2))
        atp = ctx.enter_context(tc.tile_pool(name="atp", bufs=2))
        hmup = ctx.enter_context(tc.tile_pool(name="hmup", bufs=2))
        finp = ctx.enter_context(tc.tile_pool(name="finp", bufs=2))
        orp = ctx.enter_context(tc.tile_pool(name="orp", bufs=2))
        psd2 = ctx.enter_context(tc.tile_pool(name="psd2", bufs=1, space="PSUM"))
        psacc = ctx.enter_context(tc.tile_pool(name="psacc", bufs=1, space="PSUM"))
        psout = ctx.enter_context(tc.tile_pool(name="psout", bufs=1, space="PSUM"))
        pstr = ctx.enter_context(tc.tile_pool(name="pstr", bufs=1, space="PSUM"))

        # ---- inputs, spread across engine DMA queues; critical pack first --
        dpack_sb = singles.tile([DR, BPC * NCHUNK * P + BPC * N], f16)
        nc.gpsimd.dma_start(out=dpack_sb[:], in_=dpack_d[:])
        fpack_sb = singles.tile([P, FPK], f32)
        nc.sync.dma_start(out=fpack_sb[:], in_=fpack_d[:])
        identh = singles.tile([P, 2, P], f16)  # [:,0,:]=+I, [:,1,:]=-I
        nc.sync.dma_start(out=identh[:], in_=identh_d[:])
        bpack_sb = singles.tile([P, BPK], bf16)
        nc.scalar.dma_start(out=bpack_sb[:], in_=bpack_d[:])
        xb_sb = singles.tile([DIM, BPC, N], bf16)
        nc.scalar.dma_start(out=xb_sb[:], in_=xb_d[:])
        unrep_sb = singles.tile([MU, BPC, N], f16)
        nc.sync.dma_start(out=unrep_sb[:], in_=unrep_d[:])

        # views into the packs
        SD = BPC * NCHUNK * P  # statd column count in dpack

        def statd_v(b, I):
            off = (b * NCHUNK + I) * P
            return dpack_sb[:, off : off + P]

        def xTn_v(b, j0, j1):
            off = SD + b * N
            return dpack_sb[:, off + j0 : off + j1]

        def statx6_v(b, I, c0_, c1_):
            off = (b * NCHUNK + I) * 2 * DIM
            return bpack_sb[:, off + c0_ : off + c1_]

        identb = bpack_sb[:, BPC * NCHUNK * 2 * DIM : BPC * NCHUNK * 2 * DIM + P]
        muAb = bpack_sb[0:MU, BPK - DIM : BPK]
        eab = fpack_sb[:, 0 : META + 1]
        negbeta = fpack_sb[0:MU, META + 1 : META + 2]
        c0x = fpack_sb[0:DIM, META + 2 : META + 2 + BPC]

        # ---- t strips straight from the PE (fp16 operands, 1 cyc/row) ----
        def emit_d2_chunk(b, I, tps):
            nc.tensor.matmul(
                tps[:, OFFS[I] : OFFS[I] + WIDTHS[I]],
                statd_v(b, I),
                xTn_v(b, P * I, N),
                start=True,
                stop=True,
                skip_group_check=True,
            )

        def emit_tcopies(b, tps):
            """PSUM t -> SBUF: ACT takes the A columns, DVE the B columns."""
            tA = tap.tile([P, ASPLIT], f32, tag="ta")
            nc.scalar.copy(tA[:], tps[:, 0:ASPLIT])
            tB = tbp.tile([P, NPACK - ASPLIT], f32, tag="tb")
            nc.vector.tensor_copy(tB[:], tps[:, ASPLIT:NPACK])
            return tA, tB

        def emit_expacc(b, tA, interleave=None):
            """A-region: META exp passes on ACT, +/-I fp16 accumulate on PE.
            `interleave` emits one extra PE op after each m (p-state filler)."""
            acc = psacc.tile([P, ASPLIT], f32, tag="acc")
            for m in range(META):
                hs = hsp.tile([P, ASPLIT], f16, tag="hs")
                nc.scalar.activation(
                    hs[:],
                    tA[:],
                    AF.Exp,
                    scale=ea_scale[m],
                    bias=eab[:, m : m + 1],
                )
                sgn = 0 if ea_sign[m] > 0 else 1
                for off, w in ASEGS:
                    nc.tensor.matmul(
                        acc[:, off : off + w],
                        identh[:, sgn, :],
                        hs[:, off : off + w],
                        start=(m == 0),
                        stop=(m == META - 1),
                        skip_group_check=True,
                    )
                if interleave is not None and m < len(interleave):
                    interleave[m]()
            return acc

        def emit_horner(b, tB, Mt):
            """B-region: monomial Horner for P(t) - c0 on DVE (stock ops)."""
            g = hpool.tile([P, NPACK - ASPLIT], f32, tag="h")
            nc.vector.tensor_scalar_mul(out=g[:], in0=tB[:], scalar1=pc[DEG])
            gap = g[:]
            for j in range(DEG - 1, 0, -1):
                if j == 1:
                    dst_ap = Mt[:, ASPLIT:NPACK]
                else:
                    dst = hpool.tile([P, NPACK - ASPLIT], f32, tag="h")
                    dst_ap = dst[:]
                nc.vector.scalar_tensor_tensor(
                    out=dst_ap,
                    in0=gap,
                    scalar=pc[j],
                    in1=tB[:],
                    op0=OP.add,
                    op1=OP.mult,
                )
                gap = dst_ap

        def emit_merge(b, acc, Mt):
            nc.scalar.copy(Mt[:, 0:ASPLIT], acc[:])

        def blkoff(I, J):
            return OFFS[I] + (J - I) * P

        def emit_transposes(b, Mt):
            # all 6 transposed blocks into ONE PSUM bank (ordered by J)
            tp = pstr.tile([P, NPAIR, P], bf16, tag="tr")
            for k, (I, J) in enumerate(PAIRS_BYJ):
                nc.tensor.transpose(
                    tp[:, k, :], Mt[:, blkoff(I, J) : blkoff(I, J) + P], identb
                )
            at = atp.tile([P, NPAIR, P], bf16, tag="at")
            nc.vector.tensor_copy(at[:], tp[:])
            return at

        def emit_contract(b, Mt, at):
            poutQ = psout.tile([DIM, N], f32, tag="q")
            poutP = psout.tile([DIM, N], f32, tag="p")
            NTOT = 2 * NCHUNK - 1  # merged contribution groups per tile
            ntouch = {id(poutQ): 0, id(poutP): 0}

            def contrib(out_cols, stat_chunk, stat_lo, mov_ap, tile_):
                k = ntouch[id(tile_)]
                ntouch[id(tile_)] = k + 1
                nc.tensor.matmul(
                    tile_[:, out_cols],
                    statx6_v(b, stat_chunk, stat_lo, stat_lo + DIM),
                    mov_ap,
                    start=(k == 0),
                    stop=(k == NTOT - 1 and tile_ is poutP),
                    skip_group_check=True,
                )

            # direct (incl. diagonal): stationary chunk I vs its whole strip
            for I in range(NCHUNK):
                mv = Mt[:, OFFS[I] : OFFS[I] + WIDTHS[I]]
                contrib(slice(P * I, N), I, 0, mv, poutQ)
                contrib(slice(P * I, N), I, DIM, mv, poutP)
            # reflected: stationary chunk J vs the J-group of transposed blocks
            for J in range(1, NCHUNK):
                g0 = J * (J - 1) // 2
                mv = at[:, g0 : g0 + J, :]
                contrib(slice(0, P * J), J, 0, mv, poutQ)
                contrib(slice(0, P * J), J, DIM, mv, poutP)
            # mu fold into Q rows: Q' = Q - mu - c0'  (muAb = -c' replicated)
            hmu = hmup.tile([MU, N], bf16, tag="hmu")
            nc.scalar.activation(
                hmu[:],
                unrep_sb[:, b, :],
                AF.Exp,
                scale=negbeta[:, 0:1],
                bias=eab[0:MU, META : META + 1],
            )
            nc.tensor.matmul(
                poutQ[:, :],
                muAb,
                hmu[:],
                start=False,
                stop=True,
                skip_group_check=True,
            )
            return poutQ, poutP

        def emit_finalize(b, pq):
            poutQ, poutP = pq
            # out = (P' + c0*X_c) - x*(Q' + c0*N)
            o1 = finp.tile([DIM, N], f32, tag="o1")
            nc.vector.scalar_tensor_tensor(
                out=o1[:],
                in0=poutQ[:],
                scalar=c0 * float(N),
                in1=xb_sb[:, b, :],
                op0=OP.add,
                op1=OP.mult,
            )
            outrow = orp.tile([DIM, N], f32, tag="or")
            nc.vector.scalar_tensor_tensor(
                out=outrow[:],
                in0=poutP[:],
                scalar=c0x[:, b : b + 1],
                in1=o1[:],
                op0=OP.add,
                op1=OP.subtract,
            )
            nc.gpsimd.dma_start(out=out_d[b], in_=outrow[:])

        # ---- schedule ----
        tps0 = psd2.tile([P, NPACK], f32, tag="t0")
        for I in range(NCHUNK):
            emit_d2_chunk(0, I, tps0)
        tA0, tB0 = emit_tcopies(0, tps0)
        tps1 = psd2.tile([P, NPACK], f32, tag="t0")
        # interleave b1's d^2 chunks into b0's accumulate gaps (PE stays hot)
        inter = [lambda I=I: emit_d2_chunk(1, I, tps1) for I in range(NCHUNK)]
        acc0 = emit_expacc(0, tA0, interleave=inter)
        Mt0 = mpool.tile([P, NPACK], bf16, tag="m0")
        emit_horner(0, tB0, Mt0)
        tA1, tB1 = emit_tcopies(1, tps1)
        emit_merge(0, acc0, Mt0)
        acc1 = emit_expacc(1, tA1)
        at0 = emit_transposes(0, Mt0)
        pq0 = emit_contract(0, Mt0, at0)
        Mt1 = mpool.tile([P, NPACK], bf16, tag="m1")
        emit_horner(1, tB1, Mt1)
        emit_merge(1, acc1, Mt1)
        at1 = emit_transposes(1, Mt1)
        emit_finalize(0, pq0)
        pq1 = emit_contract(1, Mt1, at1)
        emit_finalize(1, pq1)

    _spread_sync_waits(nc)
    return nc


def _ensure_ntff_hook():
    """bass_utils' axon trace path imports antenv.axon_hooks, which the image's
    antenv package lacks. Register an equivalent module backed by the boot
    package's ctypes NTFF hook so trace=True works; degrade silently if the
    pieces are missing (tracing is optional)."""
    import os
    import types

    try:
        import antenv.axon_hooks  # noqa: F401

        return
    except ImportError:
        pass
    try:
        import antenv
    except ImportError:
        return
    mod = types.ModuleType("antenv.axon_hooks")
    box = {"h": None}
    mod.set_axon_ntff_profile_hook = lambda h: box.__setitem__("h", h)
    mod.get_axon_ntff_profile_hook = lambda: box["h"]
    sys.modules["antenv.axon_hooks"] = mod
    antenv.axon_hooks = mod
    try:
        from trn_agent_boot.trn_boot import _ntff_profile_via_ctypes

        so = "/opt/axon/libaxon_pjrt.so"
        if os.path.exists(so):
            hook = _ntff_profile_via_ctypes(so)
            if hook is not None:
                mod.set_axon_ntff_profile_hook(hook)
    except Exception:
        pass


def kernel(x, eta_w1, eta_b1, eta_w2, eta_b2, mu_w1, mu_b1, mu_w2, mu_b2):
    global LAST_RESULT
    _ensure_ntff_hook()
    import ml_dtypes
    from concourse.bass_utils import run_bass_kernel_spmd

    f32 = np.float32
    f16 = np.float16
    bf = ml_dtypes.bfloat16
    x = np.ascontiguousarray(np.asarray(x, dtype=f32))
    eta_w1 = np.asarray(eta_w1, f32)
    eta_b1 = np.asarray(eta_b1, f32)
    eta_w2 = np.asarray(eta_w2, f32)
    eta_b2 = np.asarray(eta_b2, f32)
    mu_w1 = np.asarray(mu_w1, f32)
    mu_b1 = np.asarray(mu_b1, f32)
    mu_w2 = np.asarray(mu_w2, f32)
    mu_b2 = np.asarray(mu_b2, f32)

    n2_all = (x.astype(np.float64) ** 2).sum(-1)  # [B, N]
    s, pc, eta_gam, eta_ce = _fit_surrogates(x, eta_w1, eta_b1, eta_w2, eta_b2)
    mu_g, mu_c = _fit_mu_exp(n2_all, mu_w1, mu_b1, mu_w2, mu_b2)
    c0 = float(pc[0])

    nc = _build_program(pc, eta_gam, eta_ce)

    DR = DIM + 2
    identh = np.empty((P, 2, P), f16)
    identh[:, 0, :] = np.eye(P, dtype=f32)
    identh[:, 1, :] = -np.eye(P, dtype=f32)
    ea_bias = eta_gam + np.log(np.abs(eta_ce))

    BPK = BPC * NCHUNK * 2 * DIM + P + DIM
    SD = BPC * NCHUNK * P

    in_maps = []
    for core in range(NCORES):
        xc = x[core * BPC : (core + 1) * BPC]  # [BPC, N, DIM]
        xTc = xc.transpose(0, 2, 1)  # [BPC, DIM, N]
        n2 = n2_all[core * BPC : (core + 1) * BPC].astype(f32)  # [BPC, N]
        dpack = np.empty((DR, SD + BPC * N), f32)
        bpack = np.zeros((P, BPK), f32)
        for bb in range(BPC):
            xoff = SD + bb * N
            dpack[0:DIM, xoff : xoff + N] = xTc[bb]
            dpack[DIM, xoff : xoff + N] = n2[bb]
            dpack[DIM + 1, xoff : xoff + N] = 1.0
            for I in range(NCHUNK):
                soff = (bb * NCHUNK + I) * P
                dpack[0:DIM, soff : soff + P] = -2.0 * s * xTc[bb, :, I * P : (I + 1) * P]
                dpack[DIM, soff : soff + P] = s
                dpack[DIM + 1, soff : soff + P] = s * n2[bb, I * P : (I + 1) * P] - 1.0
                boff = (bb * NCHUNK + I) * 2 * DIM
                bpack[:, boff : boff + DIM] = 1.0
                bpack[:, boff + DIM : boff + 2 * DIM] = xc[bb, I * P : (I + 1) * P, :]
        bpack[:, BPC * NCHUNK * 2 * DIM : BPC * NCHUNK * 2 * DIM + P] = np.eye(P)
        bpack[0:MU, BPK - DIM : BPK] = np.repeat(-mu_c[:, None], DIM, axis=1)
        fpack = np.zeros((P, FPK), f32)
        fpack[:, 0:META] = ea_bias[None, :].astype(f32)
        fpack[0:MU, META + 1] = -mu_g
        fpack[0:DIM, META + 2 : META + 2 + BPC] = c0 * xc.sum(axis=1).T
        unrep = np.broadcast_to(n2[None, :, :], (MU, BPC, N))
        in_maps.append(
            {
                "dpack": dpack.astype(f16),
                "identh": identh,
                "bpack": bpack.astype(bf),
                "xb": np.ascontiguousarray(xTc.transpose(1, 0, 2)).astype(bf),
                "unrep": np.ascontiguousarray(unrep).astype(f16),
                "fpack": fpack,
            }
        )

    res = run_bass_kernel_spmd(nc, in_maps, core_ids=list(range(NCORES)))
    LAST_RESULT = res
    out = np.concatenate([r["out"] for r in res.results], axis=0)  # [B, DIM, N]
    return np.ascontiguousarray(out.transpose(0, 2, 1)).astype(np.float32)


# revision 39
# speedup vs baseline: 1.5769x; 1.0280x over previous
"""Trainium2 Bass kernel for the Backflow module.

Math (B=16, N=512, DIM=3, H=32):
  out[b,i,:] = sum_j eta(||x_bi - x_bj||) * (x_bi - x_bj)  +  mu(||x_bi||) * x_bi
where eta/mu are 1->H->1 tanh MLPs. The reference's eye()/diagonal correction
cancels exactly (eta(0)*(x_i - x_i) = 0 in the matrix form below).

Sharding: data-parallel over batch, 2 batches per core on 8 cores.

eta and mu are univariate scalar functions and the rel-err budget (2e-2)
is large, so we fit cheap surrogates at call time from the actual weights,
both in u = d^2 (no sqrt anywhere; exp/identity/copy live in one ACT
table set -> a single table load):

  t[i,j] = 2*d_ij^2/umax - 1 comes straight out of the PE: the d^2
  matmul carries two extra rows ([-2sx | s | s*n2_i - 1] stationary x
  [x | n2_j | 1] moving, fp16 = 1 cyc/row) so PSUM holds t directly;
  ACT copies the A-columns and DVE the B-columns to SBUF.

  M[i,j] := -eta(d_ij) - c0 evaluated two ways on disjoint column regions
  of the packed strip:
   A-region (ACT+PE): sum_m c_m exp(g_m (t+1)) - META exp ACT passes,
     |c_m| folded into the bias, sign via +/-identity fp16 stationaries
     accumulated on the PE into PSUM; one ACT copy -> bf16 M tile.
   B-region (DVE): monomial Horner for P(t) - c0 via stock
     scalar_tensor_tensor ops (g = c_deg*t; g = (g + c_j)*t).
  The split ratio load-balances ACT vs DVE.

  The shared constant c0 is folded into the finalize for free:
  out_c[j] = (P'_c[j] + c0*X_c) - x_c[j]*(Q'[j] + c0*N), X_c = sum_i x_c[i].

  mu(||x_i||) = c0' + sum_m c'_m exp(-b_m n_i^2): ONE ACT exp pass on a
  [MU, N] broadcast of n^2 (per-partition scale), folded into the Q rows
  of the PSUM contraction with a negated bf16 stationary.

Per-core layout: i on partitions (4 chunks of 128), j on the free dim.
Symmetry eta(d_ij) = eta(d_ji): compute only block-triangular strips
(chunk I covers j in [128*I, 512)), packed to [128, 1280] with
bank-aligned chunk offsets (order 0,1,3,2) so every matmul output stays
inside a PSUM bank.

Row sums via PE contractions (3-wide ones / x stationaries in bf16, M
moving in bf16 = 1 cyc/row), merged per stationary chunk: the direct
contributions of stationary chunk I cover the contiguous strip
[OFFS[I], OFFS[I]+W) -> one matmul per (I, P/Q); the reflected blocks
(via 6 PE transposes into ONE PSUM bank, one DVE copy back, ordered by
J) also merge per stationary J. Interleaving b1's d^2 matmuls into
b0's accumulate gaps keeps the PE p-state up. Input DMAs are packed
into few tensors and triggered from different engines' queues.
"""

import sys

sys.path.insert(0, "/opt/trn_rl_repo")

import numpy as np
from contextlib import ExitStack

B, N, DIM, H = 16, 512, 3, 32
NCORES = 8
BPC = B // NCORES  # batches per core
P = 128
NCHUNK = N // P  # 4
WIDTHS = [N - P * I for I in range(NCHUNK)]  # [512, 384, 256, 128]
# bank-aligned packing of the block-triangular strips (chunk order 0,1,3,2)
OFFS = [0, 512, 1024, 896]
NPACK = sum(WIDTHS)  # 1280

DEG = 8  # B-region polynomial degree
META = 5  # A-region exp basis size
MU = 12  # mu exp-basis size (incl. the g=0 constant term)
ASPLIT = 832  # packed columns [0, ASPLIT) on ACT path, rest on DVE path
ASEGS = [(0, 512), (512, 320)]  # accumulate matmul splits (PSUM banks, >=256)
assert ASEGS[-1][0] + ASEGS[-1][1] == ASPLIT

# transposed-block pairs ordered by stationary chunk J, then I;
# J-group g starts at index J*(J-1)/2 and holds J blocks (I = 0..J-1)
PAIRS_BYJ = [(I, J) for J in range(1, NCHUNK) for I in range(J)]

LAST_RESULT = None


def _spread_sync_waits(nc):
    """The pinned walrus rejects instructions carrying more than one sync wait
    ('Too many sync wait commands'). Engines execute their instruction streams
    in order, so hoist all-but-one wait of any such instruction onto same-engine
    NoOps inserted directly before it — semantically identical ordering."""
    from concourse import mybir

    n_added = 0
    for bb in nc.main_func.blocks:
        insts = bb.instructions
        i = 0
        while i < len(insts):
            inst = insts[i]
            si = getattr(inst, "sync_info", None)
            waits = list(si.on_wait) if si is not None and si.on_wait else []
            if len(waits) > 1:
                si.on_wait = waits[-1:]
                for k, w in enumerate(waits[:-1]):
                    nop = mybir.InstNoOp(
                        name=f"{inst.name}-wspread{k}",
                        sync_info=mybir.SyncInfo(on_wait=[w], on_update=[]),
                        engine=inst.engine,
                        bass_nofuse=True,
                    )
                    insts.insert(i + k, nop)
                    n_added += 1
                i += len(waits) - 1
            i += 1
    return n_added


def _eta_fn(d, w1, b1, w2, b2):
    return np.tanh(d[..., None] * w1[0] + b1) @ w2[:, 0] + b2[0]


def _fit_surrogates(x, eta_w1, eta_b1, eta_w2, eta_b2):
    """Global fits of f(t) = -eta(sqrt(u)), t = 2u/umax - 1:
    poly (ascending monomial coeffs, deg DEG) and exp basis
    f - c0 ~= sum_m c_m exp(g_m (t+1)). Returns (s, pc, gam, ce)."""
    x = x.astype(np.float64)
    n2 = (x**2).sum(-1)  # [B, N]
    rng = np.random.default_rng(0)
    umax = 0.0
    samples = []
    for b in range(B):
        G = x[b] @ x[b].T
        Ub = np.maximum(n2[b][:, None] + n2[b][None, :] - 2 * G, 0.0)
        umax = max(umax, float(Ub.max()))
        idx = rng.choice(N * N, 16384, replace=False)
        samples.append(Ub.reshape(-1)[idx])
    umax = umax * 1.002 + 1e-6
    uu = np.concatenate(samples)
    ug = np.linspace(0.0, umax, 2000)
    ufit = np.concatenate([uu, ug])
    w = np.concatenate(
        [np.sqrt(np.sqrt(uu) + 0.1), 3.0 * np.sqrt(np.sqrt(ug) + 0.1)]
    )
    tfit = 2.0 * ufit / umax - 1.0
    yfit = -_eta_fn(np.sqrt(ufit), eta_w1, eta_b1, eta_w2, eta_b2)
    import numpy.polynomial.chebyshev as Ch

    cf = Ch.chebfit(tfit, yfit, DEG, w=w)
    pc = Ch.cheb2poly(cf)  # ascending monomial coeffs in t
    c0 = float(pc[0])
    # exp basis on the residual target f - c0, no free constant
    gam = -np.geomspace(0.08, 48.0, META)  # exponents per (t+1) unit
    A = np.exp((tfit[:, None] + 1.0) * gam[None, :])
    Aw = A * w[:, None]
    ce, *_ = np.linalg.lstsq(Aw, (yfit - c0) * w, rcond=None)
    s = 2.0 / umax
    return float(s), pc.astype(np.float64), gam, ce


def _fit_mu_exp(n2_all, mu_w1, mu_b1, mu_w2, mu_b2):
    """Fit mu(sqrt(u)) ~= sum_m c_m exp(-g_m u) on the actual n^2 values
    (the exact evaluation points). g_0 = 0 supplies the constant term."""
    us = np.sort(n2_all.reshape(-1).astype(np.float64))
    n2max = float(us[-1]) * 1.001 + 1e-9
    g = np.concatenate([[0.0], np.geomspace(0.125, 96.0, MU - 1) / n2max])
    A = np.exp(-us[:, None] * g[None, :])
    y = _eta_fn(np.sqrt(us), mu_w1, mu_b1, mu_w2, mu_b2)
    w = np.sqrt(np.sqrt(us) + 0.1)
    Aw = A * w[:, None]
    AtA = Aw.T @ Aw + 1e-10 * len(us) * np.eye(MU)
    c = np.linalg.solve(AtA, Aw.T @ (y * w))
    return g.astype(np.float64), c.astype(np.float64)


# packed f32 smalls blob layout: [P, FPK] with
#   cols [0, META+1): eab (exp-basis biases + mu zero bias col)
#   col META+1: negbeta (rows 0:MU)
#   cols META+2 .. META+3: c0x (rows 0:DIM)
FPK = META + 2 + BPC


def _build_program(poly_pc, eta_gam, eta_ce):
    import concourse.bass as bass
    import concourse.tile as tile
    from concourse import mybir

    f32 = mybir.dt.float32
    f16 = mybir.dt.float16
    bf16 = mybir.dt.bfloat16
    AF = mybir.ActivationFunctionType
    OP = mybir.AluOpType

    pc = [float(v) for v in poly_pc]  # ascending, len DEG+1
    c0 = pc[0]
    ea_scale = [float(g) for g in eta_gam]
    ea_sign = [1.0 if c > 0 else -1.0 for c in eta_ce]

    DR = DIM + 2  # d^2 matmul rows: x(3), n2, ones
    NPAIR = len(PAIRS_BYJ)

    nc = bass.Bass()
    # dpack: statd [DR, BPC*NCHUNK*P] | xTn [DR, BPC*N]  (fp16, critical)
    dpack_d = nc.dram_tensor("dpack", [DR, BPC * NCHUNK * P + BPC * N], f16, kind="ExternalInput")
    identh_d = nc.dram_tensor("identh", [P, 2, P], f16, kind="ExternalInput")
    # bpack: statx6 (BPC*NCHUNK*6 cols) | identb (P cols), rows 0:MU of the
    # first 3 cols after that hold muAb
    BPK = BPC * NCHUNK * 2 * DIM + P + DIM
    bpack_d = nc.dram_tensor("bpack", [P, BPK], bf16, kind="ExternalInput")
    xb_d = nc.dram_tensor("xb", [DIM, BPC, N], bf16, kind="ExternalInput")
    unrep_d = nc.dram_tensor("unrep", [MU, BPC, N], f16, kind="ExternalInput")
    fpack_d = nc.dram_tensor("fpack", [P, FPK], f32, kind="ExternalInput")
    out_d = nc.dram_tensor("out", [BPC, DIM, N], f32, kind="ExternalOutput")

    with tile.TileContext(nc) as tc, ExitStack() as ctx:
        singles = ctx.enter_context(tc.tile_pool(name="singles", bufs=1))
        tap = ctx.enter_context(tc.tile_pool(name="tap", bufs=2))
        tbp = ctx.enter_context(tc.tile_pool(name="tbp", bufs=2))
        hpool = ctx.enter_context(tc.tile_pool(name="hpool", bufs=2))
        hsp = ctx.enter_context(tc.tile_pool(name="hsp", bufs=8))
        mpool = ctx.enter_context(tc.tile_pool(name="mpool", bufs=2))
        atp = ctx.enter_context(tc.tile_pool(name="atp", bufs=2))
        hmup = ctx.enter_context(tc.tile_pool(name="hmup", bufs=2))
        finp = ctx.enter_context(tc.tile_pool(name="finp", bufs=2))
        orp = ctx.enter_context(tc.tile_pool(name="orp", bufs=2))
        psd2 = ctx.enter_context(tc.tile_pool(name="psd2", bufs=1, space="PSUM"))
        psacc = ctx.enter_context(tc.tile_pool(name="psacc", bufs=1, space="PSUM"))
        psout = ctx.enter_context(tc.tile_pool(name="psout", bufs=1, space="PSUM"))
        pstr = ctx.enter_context(tc.tile_pool(name="pstr", bufs=1, space="PSUM"))

        # ---- inputs, spread across engine DMA queues; critical pack first --
        dpack_sb = singles.tile([DR, BPC * NCHUNK * P + BPC * N], f16)
        nc.scalar.dma_start(out=dpack_sb[:], in_=dpack_d[:])
        fpack_sb = singles.tile([P, FPK], f32)
        nc.sync.dma_start(out=fpack_sb[:], in_=fpack_d[:])
        identh = singles.tile([P, 2, P], f16)  # [:,0,:]=+I, [:,1,:]=-I
        nc.gpsimd.dma_start(out=identh[:], in_=identh_d[:])
        bpack_sb = singles.tile([P, BPK], bf16)
        nc.gpsimd.dma_start(out=bpack_sb[:], in_=bpack_d[:])
        xb_sb = singles.tile([DIM, BPC, N], bf16)
        nc.sync.dma_start(out=xb_sb[:], in_=xb_d[:])
        unrep_sb = singles.tile([MU, BPC, N], f16)
        nc.sync.dma_start(out=unrep_sb[:], in_=unrep_d[:])

        # views into the packs
        SD = BPC * NCHUNK * P  # statd column count in dpack

        def statd_v(b, I):
            off = (b * NCHUNK + I) * P
            return dpack_sb[:, off : off + P]

        def xTn_v(b, j0, j1):
            off = SD + b * N
            return dpack_sb[:, off + j0 : off + j1]

        def statx6_v(b, I, c0_, c1_):
            off = (b * NCHUNK + I) * 2 * DIM
            return bpack_sb[:, off + c0_ : off + c1_]

        identb = bpack_sb[:, BPC * NCHUNK * 2 * DIM : BPC * NCHUNK * 2 * DIM + P]
        muAb = bpack_sb[0:MU, BPK - DIM : BPK]
        eab = fpack_sb[:, 0 : META + 1]
        negbeta = fpack_sb[0:MU, META + 1 : META + 2]
        c0x = fpack_sb[0:DIM, META + 2 : META + 2 + BPC]

        # ---- t strips straight from the PE (fp16 operands, 1 cyc/row) ----
        def emit_d2_chunk(b, I, tps):
            nc.tensor.matmul(
                tps[:, OFFS[I] : OFFS[I] + WIDTHS[I]],
                statd_v(b, I),
                xTn_v(b, P * I, N),
                start=True,
                stop=True,
                skip_group_check=True,
            )

        def emit_tcopies(b, tps, skip_a=False):
            """PSUM t -> SBUF on DVE. For the last batch the A columns stay
            in PSUM (the exps read them there; nothing needs psd2 after)."""
            tA = None
            if not skip_a:
                tA = tap.tile([P, ASPLIT], f32, tag="ta")
                nc.vector.tensor_copy(tA[:], tps[:, 0:ASPLIT])
            tB = tbp.tile([P, NPACK - ASPLIT], f32, tag="tb")
            nc.vector.tensor_copy(tB[:], tps[:, ASPLIT:NPACK])
            return tA, tB

        def emit_expacc(b, t_ap, interleave=None):
            """A-region: META exp passes on ACT, +/-I fp16 accumulate on PE.
            `interleave` emits one extra PE op after each m (p-state filler)."""
            acc = psacc.tile([P, ASPLIT], f32, tag="acc")
            for m in range(META):
                hs = hsp.tile([P, ASPLIT], f16, tag="hs")
                nc.scalar.activation(
                    hs[:],
                    t_ap,
                    AF.Exp,
                    scale=ea_scale[m],
                    bias=eab[:, m : m + 1],
                )
                sgn = 0 if ea_sign[m] > 0 else 1
                for off, w in ASEGS:
                    nc.tensor.matmul(
                        acc[:, off : off + w],
                        identh[:, sgn, :],
                        hs[:, off : off + w],
                        start=(m == 0),
                        stop=(m == META - 1),
                        skip_group_check=True,
                    )
                if interleave is not None and m < len(interleave):
                    interleave[m]()
            return acc

        def emit_horner(b, tB, Mt):
            """B-region: monomial Horner for P(t) - c0 on DVE (stock ops)."""
            g = hpool.tile([P, NPACK - ASPLIT], f32, tag="h")
            nc.vector.tensor_scalar_mul(out=g[:], in0=tB[:], scalar1=pc[DEG])
            gap = g[:]
            for j in range(DEG - 1, 0, -1):
                if j == 1:
                    dst_ap = Mt[:, ASPLIT:NPACK]
                else:
                    dst = hpool.tile([P, NPACK - ASPLIT], f32, tag="h")
                    dst_ap = dst[:]
                nc.vector.scalar_tensor_tensor(
                    out=dst_ap,
                    in0=gap,
                    scalar=pc[j],
                    in1=tB[:],
                    op0=OP.add,
                    op1=OP.mult,
                )
                gap = dst_ap

        def emit_merge(b, acc, Mt):
            nc.scalar.copy(Mt[:, 0:ASPLIT], acc[:])

        def blkoff(I, J):
            return OFFS[I] + (J - I) * P

        def emit_transposes(b, Mt):
            # all 6 transposed blocks into ONE PSUM bank (ordered by J)
            tp = pstr.tile([P, NPAIR, P], bf16, tag="tr")
            for k, (I, J) in enumerate(PAIRS_BYJ):
                nc.tensor.transpose(
                    tp[:, k, :], Mt[:, blkoff(I, J) : blkoff(I, J) + P], identb
                )
            at = atp.tile([P, NPAIR, P], bf16, tag="at")
            nc.vector.tensor_copy(at[:], tp[:])
            return at

        def emit_contract(b, Mt, at):
            poutQ = psout.tile([DIM, N], f32, tag="q")
            poutP = psout.tile([DIM, N], f32, tag="p")
            NTOT = 2 * NCHUNK - 1  # merged contribution groups per tile
            ntouch = {id(poutQ): 0, id(poutP): 0}

            def contrib(out_cols, stat_chunk, stat_lo, mov_ap, tile_):
                k = ntouch[id(tile_)]
                ntouch[id(tile_)] = k + 1
                nc.tensor.matmul(
                    tile_[:, out_cols],
                    statx6_v(b, stat_chunk, stat_lo, stat_lo + DIM),
                    mov_ap,
                    start=(k == 0),
                    stop=(k == NTOT - 1 and tile_ is poutP),
                    skip_group_check=True,
                )

            # direct (incl. diagonal): stationary chunk I vs its whole strip
            for I in range(NCHUNK):
                mv = Mt[:, OFFS[I] : OFFS[I] + WIDTHS[I]]
                contrib(slice(P * I, N), I, 0, mv, poutQ)
                contrib(slice(P * I, N), I, DIM, mv, poutP)
            # reflected: stationary chunk J vs the J-group of transposed blocks
            for J in range(1, NCHUNK):
                g0 = J * (J - 1) // 2
                mv = at[:, g0 : g0 + J, :]
                contrib(slice(0, P * J), J, 0, mv, poutQ)
                contrib(slice(0, P * J), J, DIM, mv, poutP)
            # mu fold into Q rows: Q' = Q - mu - c0'  (muAb = -c' replicated)
            hmu = hmup.tile([MU, N], bf16, tag="hmu")
            nc.scalar.activation(
                hmu[:],
                unrep_sb[:, b, :],
                AF.Exp,
                scale=negbeta[:, 0:1],
                bias=eab[0:MU, META : META + 1],
            )
            nc.tensor.matmul(
                poutQ[:, :],
                muAb,
                hmu[:],
                start=False,
                stop=True,
                skip_group_check=True,
            )
            return poutQ, poutP

        def emit_finalize(b, pq):
            poutQ, poutP = pq
            # out = (P' + c0*X_c) - x*(Q' + c0*N)
            o1 = finp.tile([DIM, N], f32, tag="o1")
            nc.vector.scalar_tensor_tensor(
                out=o1[:],
                in0=poutQ[:],
                scalar=c0 * float(N),
                in1=xb_sb[:, b, :],
                op0=OP.add,
                op1=OP.mult,
            )
            outrow = orp.tile([DIM, N], f32, tag="or")
            nc.vector.scalar_tensor_tensor(
                out=outrow[:],
                in0=poutP[:],
                scalar=c0x[:, b : b + 1],
                in1=o1[:],
                op0=OP.add,
                op1=OP.subtract,
            )
            nc.gpsimd.dma_start(out=out_d[b], in_=outrow[:])

        # ---- schedule ----
        tps0 = psd2.tile([P, NPACK], f32, tag="t0")
        for I in range(NCHUNK):
            emit_d2_chunk(0, I, tps0)
        tA0, tB0 = emit_tcopies(0, tps0)
        tps1 = psd2.tile([P, NPACK], f32, tag="t0")
        # interleave b1's d^2 chunks into b0's accumulate gaps (PE stays hot)
        inter = [lambda I=I: emit_d2_chunk(1, I, tps1) for I in range(NCHUNK)]
        acc0 = emit_expacc(0, tA0[:], interleave=inter)
        Mt0 = mpool.tile([P, NPACK], bf16, tag="m0")
        emit_horner(0, tB0, Mt0)
        _, tB1 = emit_tcopies(1, tps1, skip_a=True)
        emit_merge(0, acc0, Mt0)
        acc1 = emit_expacc(1, tps1[:, 0:ASPLIT])
        at0 = emit_transposes(0, Mt0)
        pq0 = emit_contract(0, Mt0, at0)
        Mt1 = mpool.tile([P, NPACK], bf16, tag="m1")
        emit_horner(1, tB1, Mt1)
        emit_merge(1, acc1, Mt1)
        at1 = emit_transposes(1, Mt1)
        emit_finalize(0, pq0)
        pq1 = emit_contract(1, Mt1, at1)
        emit_finalize(1, pq1)

    _spread_sync_waits(nc)
    return nc


def _ensure_ntff_hook():
    """bass_utils' axon trace path imports antenv.axon_hooks, which the image's
    antenv package lacks. Register an equivalent module backed by the boot
    package's ctypes NTFF hook so trace=True works; degrade silently if the
    pieces are missing (tracing is optional)."""
    import os
    import types

    try:
        import antenv.axon_hooks  # noqa: F401

        return
    except ImportError:
        pass
    try:
        import antenv
    except ImportError:
        return
    mod = types.ModuleType("antenv.axon_hooks")
    box = {"h": None}
    mod.set_axon_ntff_profile_hook = lambda h: box.__setitem__("h", h)
    mod.get_axon_ntff_profile_hook = lambda: box["h"]
    sys.modules["antenv.axon_hooks"] = mod
    antenv.axon_hooks = mod
    try:
        from trn_agent_boot.trn_boot import _ntff_profile_via_ctypes

        so = "/opt/axon/libaxon_pjrt.so"
        if os.path.exists(so):
            hook = _ntff_profile_via_ctypes(so)
            if hook is not None:
                mod.set_axon_ntff_profile_hook(hook)
    except Exception:
        pass


def kernel(x, eta_w1, eta_b1, eta_w2, eta_b2, mu_w1, mu_b1, mu_w2, mu_b2):
    global LAST_RESULT
    _ensure_ntff_hook()
    import ml_dtypes
    from concourse.bass_utils import run_bass_kernel_spmd

    f32 = np.float32
    f16 = np.float16
    bf = ml_dtypes.bfloat16
    x = np.ascontiguousarray(np.asarray(x, dtype=f32))
    eta_w1 = np.asarray(eta_w1, f32)
    eta_b1 = np.asarray(eta_b1, f32)
    eta_w2 = np.asarray(eta_w2, f32)
    eta_b2 = np.asarray(eta_b2, f32)
    mu_w1 = np.asarray(mu_w1, f32)
    mu_b1 = np.asarray(mu_b1, f32)
    mu_w2 = np.asarray(mu_w2, f32)
    mu_b2 = np.asarray(mu_b2, f32)

    n2_all = (x.astype(np.float64) ** 2).sum(-1)  # [B, N]
    s, pc, eta_gam, eta_ce = _fit_surrogates(x, eta_w1, eta_b1, eta_w2, eta_b2)
    mu_g, mu_c = _fit_mu_exp(n2_all, mu_w1, mu_b1, mu_w2, mu_b2)
    c0 = float(pc[0])

    nc = _build_program(pc, eta_gam, eta_ce)

    DR = DIM + 2
    identh = np.empty((P, 2, P), f16)
    identh[:, 0, :] = np.eye(P, dtype=f32)
    identh[:, 1, :] = -np.eye(P, dtype=f32)
    ea_bias = eta_gam + np.log(np.abs(eta_ce))

    BPK = BPC * NCHUNK * 2 * DIM + P + DIM
    SD = BPC * NCHUNK * P

    in_maps = []
    for core in range(NCORES):
        xc = x[core * BPC : (core + 1) * BPC]  # [BPC, N, DIM]
        xTc = xc.transpose(0, 2, 1)  # [BPC, DIM, N]
        n2 = n2_all[core * BPC : (core + 1) * BPC].astype(f32)  # [BPC, N]
        dpack = np.empty((DR, SD + BPC * N), f32)
        bpack = np.zeros((P, BPK), f32)
        for bb in range(BPC):
            xoff = SD + bb * N
            dpack[0:DIM, xoff : xoff + N] = xTc[bb]
            dpack[DIM, xoff : xoff + N] = n2[bb]
            dpack[DIM + 1, xoff : xoff + N] = 1.0
            for I in range(NCHUNK):
                soff = (bb * NCHUNK + I) * P
                dpack[0:DIM, soff : soff + P] = -2.0 * s * xTc[bb, :, I * P : (I + 1) * P]
                dpack[DIM, soff : soff + P] = s
                dpack[DIM + 1, soff : soff + P] = s * n2[bb, I * P : (I + 1) * P] - 1.0
                boff = (bb * NCHUNK + I) * 2 * DIM
                bpack[:, boff : boff + DIM] = 1.0
                bpack[:, boff + DIM : boff + 2 * DIM] = xc[bb, I * P : (I + 1) * P, :]
        bpack[:, BPC * NCHUNK * 2 * DIM : BPC * NCHUNK * 2 * DIM + P] = np.eye(P)
        bpack[0:MU, BPK - DIM : BPK] = np.repeat(-mu_c[:, None], DIM, axis=1)
        fpack = np.zeros((P, FPK), f32)
        fpack[:, 0:META] = ea_bias[None, :].astype(f32)
        fpack[0:MU, META + 1] = -mu_g
        fpack[0:DIM, META + 2 : META + 2 + BPC] = c0 * xc.sum(axis=1).T
        unrep = np.broadcast_to(n2[None, :, :], (MU, BPC, N))
        in_maps.append(
            {
                "dpack": dpack.astype(f16),
                "identh": identh,
                "bpack": bpack.astype(bf),
                "xb": np.ascontiguousarray(xTc.transpose(1, 0, 2)).astype(bf),
                "unrep": np.ascontiguousarray(unrep).astype(f16),
                "fpack": fpack,
            }
        )

    res = run_bass_kernel_spmd(nc, in_maps, core_ids=list(range(NCORES)))
    LAST_RESULT = res
    out = np.concatenate([r["out"] for r in res.results], axis=0)  # [B, DIM, N]
    return np.ascontiguousarray(out.transpose(0, 2, 1)).astype(np.float32)
